# revision 15
# baseline (speedup 1.0000x reference)
"""BinaryVGG16 forward on 8 TRN2 NeuronCores, data-parallel (2 images/core).

Strategy
--------
- Layer 0 (fp32 conv 3->64): host-side im2col to [27, H*W]; single true-fp32
  PE matmul per row-pair tile; fused BN+sign via ScalarE Sign activation with
  per-channel scale/bias -> fp8 +-1 activations in a zero-padded layout.
- Layers 1..12 (binary convs): weights sign(w) in fp8, activations +-1 fp8.
  Conv = 9 shifted matmuls (one per 3x3 tap) accumulating in PSUM; channel
  blocks of 128 on partitions; optional fp8 DoubleRow pairs two 128-blocks
  per instruction. PSUM holds exact integer counts S. BN+binarize folds to
  sign(S*A + B) with A = bn_scale*alpha, B = bn_shift (fp32, host-folded),
  applied by one ScalarE pass (PSUM -> fp8 SBUF, padded layout).
- Maxpool commutes with sign(A*x+B) for A>=0, so pooling runs on the fp32
  PSUM values (ScalarE copy + two VectorE max ops) before the sign pass.
- Head: ScalarE Sign accum_out gives per-channel sums over the 7x7 map
  (= 49*mean); FC as 4 accumulated K=128 fp32 matmuls; bias+1/49 via ScalarE.

The final logits match the jax reference bitwise in emulation; on-device
differences are limited to fp32 rounding in layer 0 and the FC head.
"""

import sys

for _p in ("/opt/trn_rl_repo", "/root/.axon_site/_ro/trn_rl_repo"):
    if _p not in sys.path:
        sys.path.append(_p)

import numpy as np
from concourse import bacc, tile, mybir
from concourse.bass_utils import run_bass_kernel_spmd

F8 = mybir.dt.float8e4
F32 = mybir.dt.float32
DR = mybir.MatmulPerfMode.DoubleRow

USE_DR = False  # fp8 DoubleRow for channel-block pairs

N_CORES = 8
IMGS_PER_CORE = 2
CFG = [64, 64, 128, 128, 256, 256, 256, 512, 512, 512, 512, 512, 512]
POOL_AFTER = {1, 3, 6, 9, 12}
NUM_CLASSES = 10

# per-layer tiling: rows of conv output computed per PSUM tile
ROWS_PER_TILE = {1: 2, 2: 4, 3: 4, 4: 8, 5: 8, 6: 8, 7: 16, 8: 16, 9: 16,
                 10: 14, 11: 14, 12: 14}


def _align16(x):
    return (x + 15) & ~15


class LayerCfg:
    def __init__(self, i):
        self.i = i
        self.c_in = 3 if i == 0 else CFG[i - 1]
        self.c_out = CFG[i]
        # input spatial (= conv output spatial)
        h = 224
        for j in range(i):
            if j in POOL_AFTER:
                h //= 2
        self.h = h
        self.pw = h + 2
        self.s = (h + 2) * self.pw
        self.bs = _align16(self.s + 32)  # block stride, data at +16
        self.kg = max(1, self.c_in // 128)
        self.p_in = min(self.c_in, 128)
        self.mt = (self.c_out + 127) // 128
        self.pooled = i in POOL_AFTER
        self.h_out = h // 2 if self.pooled else h
        self.pw_out = self.h_out + 2
        self.bs_out = _align16((self.h_out + 2) * self.pw_out + 32)


LC = [LayerCfg(i) for i in range(13)]


# ---------------------------------------------------------------------------
# host-side parameter preparation
# ---------------------------------------------------------------------------

def _prep_params(params):
    f8np = mybir.dt.np(F8)
    out = {}
    for i in range(13):
        w = np.asarray(params["convs"][i], np.float32)
        g, b, m, v = [np.asarray(t, np.float32) for t in params["bns"][i]]
        scale = g / np.sqrt(v + np.float32(1e-5))
        shift = b - m * scale
        if i == 0:
            # lhsT [27, 64]: k = c*9 + dy*3 + dx
            lhsT = w.transpose(1, 2, 3, 0).reshape(27, 64).astype(np.float32)
            out["w0"] = lhsT
            out["ab0"] = np.stack([scale, shift])
        else:
            lc = LC[i]
            alpha = np.mean(np.abs(w), axis=(1, 2, 3), dtype=np.float32)
            A = scale * np.minimum(alpha, np.float32(1.0))
            assert np.all(A >= 0), "pool/sign commute needs A >= 0"
            out[f"ab{i}"] = np.stack([A, shift])
            ws = np.sign(w).astype(np.float32)  # [O, C, 3, 3]
            C, O = lc.c_in, lc.c_out
            ws = ws.transpose(1, 2, 3, 0).reshape(C, 9, O)  # [C, t, O]
            if lc.c_in >= 128:
                wp = ws.reshape(lc.kg, 128, 9, O).transpose(1, 2, 0, 3)
            else:
                wp = ws.reshape(1, C, 9, O).transpose(1, 2, 0, 3)
            # [P, 9, KG, O]
            out[f"w{i}"] = np.ascontiguousarray(wp).astype(f8np)
    fc_w = np.asarray(params["fc_w"], np.float32)  # [10, 512]
    fc_b = np.asarray(params["fc_b"], np.float32)
    out["fcw"] = np.ascontiguousarray(
        fc_w.T.reshape(4, 128, NUM_CLASSES).transpose(1, 0, 2)
    )  # [128, 4, 10]
    out["fcb"] = fc_b
    return out


def _im2col(x):
    n = x.shape[0]
    xp = np.zeros((n, 3, 226, 226), np.float32)
    xp[:, :, 1:225, 1:225] = x
    im = np.empty((n, 27, 224 * 224), np.float32)
    for c in range(3):
        for dy in range(3):
            for dx in range(3):
                im[:, c * 9 + dy * 3 + dx] = xp[
                    :, c, dy : dy + 224, dx : dx + 224
                ].reshape(n, -1)
    return im


# ---------------------------------------------------------------------------
# device program
# ---------------------------------------------------------------------------

def _emit_borders(nc, buf, lc_out_pw, h_out, bs, kg):
    """Zero the pad ring of one act buffer (data at offset +16)."""
    pw = lc_out_pw
    for g in range(kg):
        # front pad + row0 + row1's left col
        nc.vector.memset(buf[:, g, 0 : 16 + pw + 1], 0.0)
        # (right col of row r, left col of row r+1) pairs, r = 1..h_out-1
        if h_out > 1:
            v = buf[:, g, 16 + 2 * pw - 1 : 16 + (h_out + 1) * pw - 1]
            v = v.rearrange("p (r c) -> p r c", r=h_out - 1, c=pw)
            nc.vector.memset(v[:, :, 0:2], 0.0)
        # last row's right col + bottom row + tail pad
        nc.vector.memset(buf[:, g, 16 + (h_out + 2) * pw - pw - 1 : bs], 0.0)


def build_nc():
    nc = bacc.Bacc("TRN2")
    d_x0 = nc.declare_dram_parameter(
        "x0", [IMGS_PER_CORE, 27, 224 * 224], F32, isOutput=False
    )
    d_w = {0: nc.declare_dram_parameter("w0", [27, 64], F32, isOutput=False)}
    d_ab = {}
    for i in range(13):
        lc = LC[i]
        if i > 0:
            d_w[i] = nc.declare_dram_parameter(
                f"w{i}", [lc.p_in, 9, lc.kg, lc.c_out], F8, isOutput=False
            )
        d_ab[i] = nc.declare_dram_parameter(
            f"ab{i}", [2, lc.c_out], F32, isOutput=False
        )
    d_fcw = nc.declare_dram_parameter("fcw", [128, 4, NUM_CLASSES], F32,
                                      isOutput=False)
    d_fcb = nc.declare_dram_parameter("fcb", [NUM_CLASSES], F32, isOutput=False)
    d_out = nc.declare_dram_parameter("out", [IMGS_PER_CORE, NUM_CLASSES], F32,
                                      isOutput=True)

    with tile.TileContext(nc) as tc:
        with (
            tc.tile_pool(name="wpool", bufs=2) as wpool,
            tc.tile_pool(name="abpool", bufs=1) as abpool,
            tc.tile_pool(name="iopool", bufs=4) as iopool,
            tc.tile_pool(name="tmppool", bufs=4) as tmppool,
            tc.tile_pool(name="pspool", bufs=8, space="PSUM") as pspool,
        ):
            # per-channel A/B columns, persistent (tiny)
            ab_sb = {}
            for i in range(13):
                lc = LC[i]
                t = abpool.tile([min(128, lc.c_out), 2 * lc.mt], F32,
                                name=f"ab_sb{i}")
                for m in range(lc.mt):
                    sl = slice(128 * m, 128 * m + min(128, lc.c_out - 128 * m))
                    nc.sync.dma_start(out=t[:, 2 * m], in_=d_ab[i][0, sl])
                    nc.sync.dma_start(out=t[:, 2 * m + 1], in_=d_ab[i][1, sl])
                ab_sb[i] = t
            fcw_sb = abpool.tile([128, 4, NUM_CLASSES], F32, name="fcw_sb")
            nc.sync.dma_start(out=fcw_sb[:], in_=d_fcw[:])
            fcb_sb = abpool.tile([NUM_CLASSES, 1], F32, name="fcb_sb")
            nc.sync.dma_start(out=fcb_sb[:, 0], in_=d_fcb[:])
            w0_sb = abpool.tile([27, 64], F32, name="w0_sb")
            nc.sync.dma_start(out=w0_sb[:], in_=d_w[0][:])

            for img in range(IMGS_PER_CORE):
                with tc.tile_pool(name=f"acts{img}", bufs=1) as acts:
                    bufs = {}
                    for i in range(13):
                        lc = LC[i]
                        if i < 12:
                            nxt = LC[i + 1]
                            bufs[i] = acts.tile(
                                [nxt.p_in, nxt.kg, nxt.bs], F8,
                                name=f"act{img}_{i}",
                            )
                            _emit_borders(nc, bufs[i], nxt.pw, nxt.h, nxt.bs,
                                          nxt.kg)

                    # ---- layer 0: fp32 im2col conv + BN + sign
                    lc = LC[0]
                    out_buf = bufs[0]
                    for r0 in range(0, 224, 2):
                        rhs = iopool.tile([27, 448], F32, tag="l0rhs", bufs=3)
                        nc.sync.dma_start(
                            out=rhs[:],
                            in_=d_x0[img, :, r0 * 224 : (r0 + 2) * 224],
                        )
                        ps = pspool.tile([64, 512], F32, tag="ps", bufs=8)
                        nc.tensor.matmul(ps[:, 0:448], w0_sb[:], rhs[:],
                                         start=True, stop=True)
                        st = 16 + (r0 + 1) * 226 + 1
                        dst = out_buf[:, 0, st : st + 2 * 226]
                        dst = dst.rearrange("p (r c) -> p r c", r=2, c=226)
                        nc.scalar.activation(
                            dst[:, :, 0:224],
                            ps[:, 0:448].rearrange("p (r c) -> p r c", r=2,
                                                   c=224),
                            mybir.ActivationFunctionType.Sign,
                            bias=ab_sb[0][:, 1:2],
                            scale=ab_sb[0][:, 0:1],
                        )

                    # ---- layers 1..12: binary convs
                    for i in range(1, 13):
                        lc = LC[i]
                        w_t = wpool.tile([lc.p_in, 9, lc.kg, lc.c_out], F8,
                                         tag="w", name=f"w_sb{img}_{i}")
                        nc.sync.dma_start(out=w_t[:], in_=d_w[i][:])
                        in_buf = bufs[i - 1]
                        out_buf = bufs[i] if i < 12 else None
                        feat = None
                        if i == 12:
                            feat = tmppool.tile([128, 4], F32, tag="feat",
                                                name=f"feat{img}")
                        rpt = ROWS_PER_TILE[i]
                        taps = [(dy, dx) for dy in (-1, 0, 1)
                                for dx in (-1, 0, 1)]
                        r0 = 0
                        while r0 < lc.h:
                            rr = min(rpt, lc.h - r0)
                            f0 = (r0 + 1) * lc.pw
                            n = rr * lc.pw
                            for m in range(lc.mt):
                                mw = min(128, lc.c_out - 128 * m)
                                msl = slice(128 * m, 128 * m + mw)
                                ps = pspool.tile([mw, 512], F32, tag="ps",
                                                 bufs=8)
                                nmm = 0
                                tot = 9 * lc.kg
                                for (dy, dx) in taps:
                                    off = dy * lc.pw + dx
                                    base = 16 + f0 + off
                                    kg = 0
                                    while kg < lc.kg:
                                        two = (USE_DR and lc.kg - kg >= 2)
                                        if two:
                                            nc.tensor.matmul(
                                                ps[:, 0:n],
                                                w_t[:, 3 * (dy + 1) + dx + 1,
                                                    kg : kg + 2, msl],
                                                in_buf[:, kg : kg + 2,
                                                       base : base + n],
                                                start=(nmm == 0),
                                                stop=(nmm + 2 == tot),
                                                perf_mode=DR,
                                            )
                                            nmm += 2
                                            kg += 2
                                        else:
                                            nc.tensor.matmul(
                                                ps[:, 0:n],
                                                w_t[:, 3 * (dy + 1) + dx + 1,
                                                    kg, msl],
                                                in_buf[:, kg, base : base + n],
                                                start=(nmm == 0),
                                                stop=(nmm + 1 == tot),
                                            )
                                            nmm += 1
                                            kg += 1
                                if lc.pooled:
                                    # 2x2 maxpool on fp32 PSUM, then sign
                                    half = rr // 2
                                    pse = ps[:, 0:n].rearrange(
                                        "p (r c) -> p r c", r=rr, c=lc.pw)
                                    even = tmppool.tile(
                                        [mw, half * lc.pw], F32, tag="poolA",
                                        bufs=3)
                                    nc.scalar.copy(
                                        even[:].rearrange(
                                            "p (r c) -> p r c", r=half,
                                            c=lc.pw),
                                        pse[:, 0 : rr : 2, :])
                                    rmax = tmppool.tile(
                                        [mw, half * lc.pw], F32, tag="poolB",
                                        bufs=3)
                                    nc.vector.tensor_tensor(
                                        rmax[:].rearrange(
                                            "p (r c) -> p r c", r=half,
                                            c=lc.pw),
                                        even[:].rearrange(
                                            "p (r c) -> p r c", r=half,
                                            c=lc.pw),
                                        pse[:, 1 : rr : 2, :],
                                        mybir.AluOpType.max)
                                    rm3 = rmax[:].rearrange(
                                        "p (r c) -> p r c", r=half, c=lc.pw)
                                    w2 = lc.h // 2
                                    cmax = tmppool.tile(
                                        [mw, half * w2], F32, tag="poolC",
                                        bufs=3)
                                    nc.vector.tensor_tensor(
                                        cmax[:].rearrange(
                                            "p (r c) -> p r c", r=half, c=w2),
                                        rm3[:, :, 1 : 1 + 2 * w2 : 2],
                                        rm3[:, :, 2 : 2 + 2 * w2 : 2],
                                        mybir.AluOpType.max)
                                    if i == 12:
                                        scratch = tmppool.tile(
                                            [mw, 49], F32, tag="sc12", bufs=2)
                                        nc.scalar.activation(
                                            scratch[:], cmax[:],
                                            mybir.ActivationFunctionType.Sign,
                                            bias=ab_sb[i][:, 2 * m + 1 :
                                                          2 * m + 2],
                                            scale=ab_sb[i][:, 2 * m :
                                                           2 * m + 1],
                                            accum_out=feat[:, m : m + 1])
                                    else:
                                        pw2 = lc.pw_out
                                        st = 16 + (r0 // 2 + 1) * pw2 + 1
                                        dst = out_buf[:, m,
                                                      st : st + half * pw2]
                                        nc.scalar.activation(
                                            dst.rearrange(
                                                "p (r c) -> p r c", r=half,
                                                c=pw2)[:, :, 0:w2],
                                            cmax[:].rearrange(
                                                "p (r c) -> p r c", r=half,
                                                c=w2),
                                            mybir.ActivationFunctionType.Sign,
                                            bias=ab_sb[i][:, 2 * m + 1 :
                                                          2 * m + 2],
                                            scale=ab_sb[i][:, 2 * m :
                                                           2 * m + 1])
                                else:
                                    # interior-only write (borders stay zero)
                                    st = 16 + f0 + 1
                                    dst = out_buf[:, m, st : st + rr * lc.pw]
                                    dst = dst.rearrange(
                                        "p (r c) -> p r c", r=rr, c=lc.pw)
                                    src = ps[:, 0:n].rearrange(
                                        "p (r c) -> p r c", r=rr, c=lc.pw)
                                    nc.scalar.activation(
                                        dst[:, :, 0 : lc.h],
                                        src[:, :, 1 : 1 + lc.h],
                                        mybir.ActivationFunctionType.Sign,
                                        bias=ab_sb[i][:, 2 * m + 1 : 2 * m + 2],
                                        scale=ab_sb[i][:, 2 * m : 2 * m + 1])
                            r0 += rr

                    # ---- head: FC on accumulated sign sums (feat = 49*mean)
                    ps_fc = pspool.tile([NUM_CLASSES, 512], F32, tag="ps",
                                        bufs=8)
                    for kg in range(4):
                        nc.tensor.matmul(
                            ps_fc[:, 0:1], fcw_sb[:, kg, :],
                            feat[:, kg : kg + 1],
                            start=(kg == 0), stop=(kg == 3))
                    logits = tmppool.tile([NUM_CLASSES, 1], F32, tag="logits",
                                          bufs=2)
                    nc.scalar.activation(
                        logits[:], ps_fc[:, 0:1],
                        mybir.ActivationFunctionType.Identity,
                        bias=fcb_sb[:], scale=float(np.float32(1.0 / 49.0)))
                    nc.sync.dma_start(out=d_out[img, :], in_=logits[:, 0])

    nc.finalize()
    return nc


_NC_CACHE = None


def kernel(x, params):
    global _NC_CACHE
    x = np.asarray(x, np.float32)
    prep = _prep_params(params)
    im = _im2col(x)
    if _NC_CACHE is None:
        _NC_CACHE = build_nc()
    nc = _NC_CACHE
    in_maps = []
    for core in range(N_CORES):
        m = dict(prep)
        m["x0"] = np.ascontiguousarray(
            im[core * IMGS_PER_CORE : (core + 1) * IMGS_PER_CORE])
        in_maps.append(m)
    res = run_bass_kernel_spmd(nc, in_maps, list(range(N_CORES))).results
    return np.concatenate([r["out"] for r in res], axis=0)


# revision 16
# speedup vs baseline: 1.1400x; 1.1400x over previous
"""BinaryVGG16 forward on 8 TRN2 NeuronCores, data-parallel (2 images/core).

Strategy
--------
- Layer 0 (fp32 conv 3->64): host-side im2col to [27, H*W]; single true-fp32
  PE matmul per row-pair tile; fused BN+sign via ScalarE Sign activation with
  per-channel scale/bias -> fp8 +-1 activations in a zero-padded layout.
- Layers 1..12 (binary convs): weights sign(w) in fp8, activations +-1 fp8.
  Conv = 9 shifted matmuls (one per 3x3 tap) accumulating in PSUM; channel
  blocks of 128 on partitions; optional fp8 DoubleRow pairs two 128-blocks
  per instruction. PSUM holds exact integer counts S. BN+binarize folds to
  sign(S*A + B) with A = bn_scale*alpha, B = bn_shift (fp32, host-folded),
  applied by one ScalarE pass (PSUM -> fp8 SBUF, padded layout).
- Maxpool commutes with sign(A*x+B) for A>=0, so pooling runs on the fp32
  PSUM values (ScalarE copy + two VectorE max ops) before the sign pass.
- Head: ScalarE Sign accum_out gives per-channel sums over the 7x7 map
  (= 49*mean); FC as 4 accumulated K=128 fp32 matmuls; bias+1/49 via ScalarE.

The final logits match the jax reference bitwise in emulation; on-device
differences are limited to fp32 rounding in layer 0 and the FC head.
"""

import sys

for _p in ("/opt/trn_rl_repo", "/root/.axon_site/_ro/trn_rl_repo"):
    if _p not in sys.path:
        sys.path.append(_p)

import numpy as np
from concourse import bacc, tile, mybir
from concourse.bass_utils import run_bass_kernel_spmd

F8 = mybir.dt.float8e4
F32 = mybir.dt.float32
DR = mybir.MatmulPerfMode.DoubleRow

USE_DR = True  # fp8 DoubleRow for channel-block pairs

N_CORES = 8
IMGS_PER_CORE = 2
CFG = [64, 64, 128, 128, 256, 256, 256, 512, 512, 512, 512, 512, 512]
POOL_AFTER = {1, 3, 6, 9, 12}
NUM_CLASSES = 10

# per-layer tiling: rows of conv output computed per PSUM tile
ROWS_PER_TILE = {1: 2, 2: 4, 3: 4, 4: 8, 5: 8, 6: 8, 7: 16, 8: 16, 9: 16,
                 10: 14, 11: 14, 12: 14}


def _align16(x):
    return (x + 15) & ~15


class LayerCfg:
    def __init__(self, i):
        self.i = i
        self.c_in = 3 if i == 0 else CFG[i - 1]
        self.c_out = CFG[i]
        # input spatial (= conv output spatial)
        h = 224
        for j in range(i):
            if j in POOL_AFTER:
                h //= 2
        self.h = h
        self.pw = h + 2
        self.s = (h + 2) * self.pw
        self.bs = _align16(self.s + 32)  # block stride, data at +16
        self.kg = max(1, self.c_in // 128)
        self.p_in = min(self.c_in, 128)
        self.mt = (self.c_out + 127) // 128
        self.pooled = i in POOL_AFTER
        self.h_out = h // 2 if self.pooled else h
        self.pw_out = self.h_out + 2
        self.bs_out = _align16((self.h_out + 2) * self.pw_out + 32)


LC = [LayerCfg(i) for i in range(13)]


# ---------------------------------------------------------------------------
# host-side parameter preparation
# ---------------------------------------------------------------------------

def _prep_params(params):
    f8np = mybir.dt.np(F8)
    out = {}
    for i in range(13):
        w = np.asarray(params["convs"][i], np.float32)
        g, b, m, v = [np.asarray(t, np.float32) for t in params["bns"][i]]
        scale = g / np.sqrt(v + np.float32(1e-5))
        shift = b - m * scale
        if i == 0:
            # lhsT [27, 64]: k = c*9 + dy*3 + dx
            lhsT = w.transpose(1, 2, 3, 0).reshape(27, 64).astype(np.float32)
            out["w0"] = lhsT
            out["ab0"] = np.stack([scale, shift])
        else:
            lc = LC[i]
            alpha = np.mean(np.abs(w), axis=(1, 2, 3), dtype=np.float32)
            A = scale * np.minimum(alpha, np.float32(1.0))
            assert np.all(A >= 0), "pool/sign commute needs A >= 0"
            out[f"ab{i}"] = np.stack([A, shift])
            ws = np.sign(w).astype(np.float32)  # [O, C, 3, 3]
            C, O = lc.c_in, lc.c_out
            ws = ws.transpose(1, 2, 3, 0).reshape(C, 9, O)  # [C, t, O]
            if lc.c_in >= 128:
                wp = ws.reshape(lc.kg, 128, 9, O).transpose(1, 2, 0, 3)
            else:
                wp = ws.reshape(1, C, 9, O).transpose(1, 2, 0, 3)
            # [P, 9, KG, O]
            out[f"w{i}"] = np.ascontiguousarray(wp).astype(f8np)
    fc_w = np.asarray(params["fc_w"], np.float32)  # [10, 512]
    fc_b = np.asarray(params["fc_b"], np.float32)
    out["fcw"] = np.ascontiguousarray(
        fc_w.T.reshape(4, 128, NUM_CLASSES).transpose(1, 0, 2)
    )  # [128, 4, 10]
    out["fcb"] = fc_b
    return out


def _im2col(x):
    n = x.shape[0]
    xp = np.zeros((n, 3, 226, 226), np.float32)
    xp[:, :, 1:225, 1:225] = x
    im = np.empty((n, 27, 224 * 224), np.float32)
    for c in range(3):
        for dy in range(3):
            for dx in range(3):
                im[:, c * 9 + dy * 3 + dx] = xp[
                    :, c, dy : dy + 224, dx : dx + 224
                ].reshape(n, -1)
    return im


# ---------------------------------------------------------------------------
# device program
# ---------------------------------------------------------------------------

def _emit_borders(nc, buf, lc_out_pw, h_out, bs, kg):
    """Zero the pad ring of one act buffer (data at offset +16)."""
    pw = lc_out_pw
    for g in range(kg):
        # front pad + row0 + row1's left col
        nc.vector.memset(buf[:, g, 0 : 16 + pw + 1], 0.0)
        # (right col of row r, left col of row r+1) pairs, r = 1..h_out-1
        if h_out > 1:
            v = buf[:, g, 16 + 2 * pw - 1 : 16 + (h_out + 1) * pw - 1]
            v = v.rearrange("p (r c) -> p r c", r=h_out - 1, c=pw)
            nc.vector.memset(v[:, :, 0:2], 0.0)
        # last row's right col + bottom row + tail pad
        nc.vector.memset(buf[:, g, 16 + (h_out + 2) * pw - pw - 1 : bs], 0.0)


def build_nc():
    nc = bacc.Bacc("TRN2")
    d_x0 = nc.declare_dram_parameter(
        "x0", [IMGS_PER_CORE, 27, 224 * 224], F32, isOutput=False
    )
    d_w = {0: nc.declare_dram_parameter("w0", [27, 64], F32, isOutput=False)}
    d_ab = {}
    for i in range(13):
        lc = LC[i]
        if i > 0:
            d_w[i] = nc.declare_dram_parameter(
                f"w{i}", [lc.p_in, 9, lc.kg, lc.c_out], F8, isOutput=False
            )
        d_ab[i] = nc.declare_dram_parameter(
            f"ab{i}", [2, lc.c_out], F32, isOutput=False
        )
    d_fcw = nc.declare_dram_parameter("fcw", [128, 4, NUM_CLASSES], F32,
                                      isOutput=False)
    d_fcb = nc.declare_dram_parameter("fcb", [NUM_CLASSES], F32, isOutput=False)
    d_out = nc.declare_dram_parameter("out", [IMGS_PER_CORE, NUM_CLASSES], F32,
                                      isOutput=True)

    with tile.TileContext(nc) as tc:
        with (
            tc.tile_pool(name="wpool", bufs=2) as wpool,
            tc.tile_pool(name="abpool", bufs=1) as abpool,
            tc.tile_pool(name="iopool", bufs=4) as iopool,
            tc.tile_pool(name="tmppool", bufs=4) as tmppool,
            tc.tile_pool(name="pspool", bufs=8, space="PSUM") as pspool,
        ):
            # per-channel A/B columns, persistent (tiny)
            ab_sb = {}
            for i in range(13):
                lc = LC[i]
                t = abpool.tile([min(128, lc.c_out), 2 * lc.mt], F32,
                                name=f"ab_sb{i}")
                for m in range(lc.mt):
                    sl = slice(128 * m, 128 * m + min(128, lc.c_out - 128 * m))
                    nc.sync.dma_start(out=t[:, 2 * m], in_=d_ab[i][0, sl])
                    nc.sync.dma_start(out=t[:, 2 * m + 1], in_=d_ab[i][1, sl])
                ab_sb[i] = t
            fcw_sb = abpool.tile([128, 4, NUM_CLASSES], F32, name="fcw_sb")
            nc.sync.dma_start(out=fcw_sb[:], in_=d_fcw[:])
            fcb_sb = abpool.tile([NUM_CLASSES, 1], F32, name="fcb_sb")
            nc.sync.dma_start(out=fcb_sb[:, 0], in_=d_fcb[:])
            w0_sb = abpool.tile([27, 64], F32, name="w0_sb")
            nc.sync.dma_start(out=w0_sb[:], in_=d_w[0][:])

            for img in range(IMGS_PER_CORE):
                with tc.tile_pool(name=f"acts{img}", bufs=1) as acts:
                    bufs = {}
                    for i in range(13):
                        lc = LC[i]
                        if i < 12:
                            nxt = LC[i + 1]
                            bufs[i] = acts.tile(
                                [nxt.p_in, nxt.kg, nxt.bs], F8,
                                name=f"act{img}_{i}",
                            )
                            _emit_borders(nc, bufs[i], nxt.pw, nxt.h, nxt.bs,
                                          nxt.kg)

                    # ---- layer 0: fp32 im2col conv + BN + sign
                    lc = LC[0]
                    out_buf = bufs[0]
                    for r0 in range(0, 224, 2):
                        rhs = iopool.tile([27, 448], F32, tag="l0rhs", bufs=3)
                        nc.sync.dma_start(
                            out=rhs[:],
                            in_=d_x0[img, :, r0 * 224 : (r0 + 2) * 224],
                        )
                        ps = pspool.tile([64, 512], F32, tag="ps", bufs=8)
                        nc.tensor.matmul(ps[:, 0:448], w0_sb[:], rhs[:],
                                         start=True, stop=True)
                        st = 16 + (r0 + 1) * 226 + 1
                        dst = out_buf[:, 0, st : st + 2 * 226]
                        dst = dst.rearrange("p (r c) -> p r c", r=2, c=226)
                        nc.scalar.activation(
                            dst[:, :, 0:224],
                            ps[:, 0:448].rearrange("p (r c) -> p r c", r=2,
                                                   c=224),
                            mybir.ActivationFunctionType.Sign,
                            bias=ab_sb[0][:, 1:2],
                            scale=ab_sb[0][:, 0:1],
                        )

                    # ---- layers 1..12: binary convs
                    for i in range(1, 13):
                        lc = LC[i]
                        w_t = wpool.tile([lc.p_in, 9, lc.kg, lc.c_out], F8,
                                         tag="w", name=f"w_sb{img}_{i}")
                        nc.sync.dma_start(out=w_t[:], in_=d_w[i][:])
                        in_buf = bufs[i - 1]
                        out_buf = bufs[i] if i < 12 else None
                        feat = None
                        if i == 12:
                            feat = tmppool.tile([128, 4], F32, tag="feat",
                                                name=f"feat{img}")
                        rpt = ROWS_PER_TILE[i]
                        taps = [(dy, dx) for dy in (-1, 0, 1)
                                for dx in (-1, 0, 1)]
                        r0 = 0
                        while r0 < lc.h:
                            rr = min(rpt, lc.h - r0)
                            f0 = (r0 + 1) * lc.pw
                            n = rr * lc.pw
                            for m in range(lc.mt):
                                mw = min(128, lc.c_out - 128 * m)
                                msl = slice(128 * m, 128 * m + mw)
                                ps = pspool.tile([mw, 512], F32, tag="ps",
                                                 bufs=8)
                                nmm = 0
                                tot = 9 * lc.kg
                                for (dy, dx) in taps:
                                    off = dy * lc.pw + dx
                                    base = 16 + f0 + off
                                    kg = 0
                                    while kg < lc.kg:
                                        two = (USE_DR and lc.kg - kg >= 2)
                                        if two:
                                            nc.tensor.matmul(
                                                ps[:, 0:n],
                                                w_t[:, 3 * (dy + 1) + dx + 1,
                                                    kg : kg + 2, msl],
                                                in_buf[:, kg : kg + 2,
                                                       base : base + n],
                                                start=(nmm == 0),
                                                stop=(nmm + 2 == tot),
                                                perf_mode=DR,
                                            )
                                            nmm += 2
                                            kg += 2
                                        else:
                                            nc.tensor.matmul(
                                                ps[:, 0:n],
                                                w_t[:, 3 * (dy + 1) + dx + 1,
                                                    kg, msl],
                                                in_buf[:, kg, base : base + n],
                                                start=(nmm == 0),
                                                stop=(nmm + 1 == tot),
                                            )
                                            nmm += 1
                                            kg += 1
                                if lc.pooled:
                                    # 2x2 maxpool on fp32 PSUM, then sign
                                    half = rr // 2
                                    pse = ps[:, 0:n].rearrange(
                                        "p (r c) -> p r c", r=rr, c=lc.pw)
                                    even = tmppool.tile(
                                        [mw, half * lc.pw], F32, tag="poolA",
                                        bufs=3)
                                    nc.scalar.copy(
                                        even[:].rearrange(
                                            "p (r c) -> p r c", r=half,
                                            c=lc.pw),
                                        pse[:, 0 : rr : 2, :])
                                    rmax = tmppool.tile(
                                        [mw, half * lc.pw], F32, tag="poolB",
                                        bufs=3)
                                    nc.vector.tensor_tensor(
                                        rmax[:].rearrange(
                                            "p (r c) -> p r c", r=half,
                                            c=lc.pw),
                                        even[:].rearrange(
                                            "p (r c) -> p r c", r=half,
                                            c=lc.pw),
                                        pse[:, 1 : rr : 2, :],
                                        mybir.AluOpType.max)
                                    rm3 = rmax[:].rearrange(
                                        "p (r c) -> p r c", r=half, c=lc.pw)
                                    w2 = lc.h // 2
                                    cmax = tmppool.tile(
                                        [mw, half * w2], F32, tag="poolC",
                                        bufs=3)
                                    nc.vector.tensor_tensor(
                                        cmax[:].rearrange(
                                            "p (r c) -> p r c", r=half, c=w2),
                                        rm3[:, :, 1 : 1 + 2 * w2 : 2],
                                        rm3[:, :, 2 : 2 + 2 * w2 : 2],
                                        mybir.AluOpType.max)
                                    if i == 12:
                                        scratch = tmppool.tile(
                                            [mw, 49], F32, tag="sc12", bufs=2)
                                        nc.scalar.activation(
                                            scratch[:], cmax[:],
                                            mybir.ActivationFunctionType.Sign,
                                            bias=ab_sb[i][:, 2 * m + 1 :
                                                          2 * m + 2],
                                            scale=ab_sb[i][:, 2 * m :
                                                           2 * m + 1],
                                            accum_out=feat[:, m : m + 1])
                                    else:
                                        pw2 = lc.pw_out
                                        st = 16 + (r0 // 2 + 1) * pw2 + 1
                                        dst = out_buf[:, m,
                                                      st : st + half * pw2]
                                        nc.scalar.activation(
                                            dst.rearrange(
                                                "p (r c) -> p r c", r=half,
                                                c=pw2)[:, :, 0:w2],
                                            cmax[:].rearrange(
                                                "p (r c) -> p r c", r=half,
                                                c=w2),
                                            mybir.ActivationFunctionType.Sign,
                                            bias=ab_sb[i][:, 2 * m + 1 :
                                                          2 * m + 2],
                                            scale=ab_sb[i][:, 2 * m :
                                                           2 * m + 1])
                                else:
                                    # interior-only write (borders stay zero)
                                    st = 16 + f0 + 1
                                    dst = out_buf[:, m, st : st + rr * lc.pw]
                                    dst = dst.rearrange(
                                        "p (r c) -> p r c", r=rr, c=lc.pw)
                                    src = ps[:, 0:n].rearrange(
                                        "p (r c) -> p r c", r=rr, c=lc.pw)
                                    nc.scalar.activation(
                                        dst[:, :, 0 : lc.h],
                                        src[:, :, 1 : 1 + lc.h],
                                        mybir.ActivationFunctionType.Sign,
                                        bias=ab_sb[i][:, 2 * m + 1 : 2 * m + 2],
                                        scale=ab_sb[i][:, 2 * m : 2 * m + 1])
                            r0 += rr

                    # ---- head: FC on accumulated sign sums (feat = 49*mean)
                    ps_fc = pspool.tile([NUM_CLASSES, 512], F32, tag="ps",
                                        bufs=8)
                    for kg in range(4):
                        nc.tensor.matmul(
                            ps_fc[:, 0:1], fcw_sb[:, kg, :],
                            feat[:, kg : kg + 1],
                            start=(kg == 0), stop=(kg == 3))
                    logits = tmppool.tile([NUM_CLASSES, 1], F32, tag="logits",
                                          bufs=2)
                    nc.scalar.activation(
                        logits[:], ps_fc[:, 0:1],
                        mybir.ActivationFunctionType.Identity,
                        bias=fcb_sb[:], scale=float(np.float32(1.0 / 49.0)))
                    nc.sync.dma_start(out=d_out[img, :], in_=logits[:, 0])

    nc.finalize()
    return nc


_NC_CACHE = None


def kernel(x, params):
    global _NC_CACHE
    x = np.asarray(x, np.float32)
    prep = _prep_params(params)
    im = _im2col(x)
    if _NC_CACHE is None:
        _NC_CACHE = build_nc()
    nc = _NC_CACHE
    in_maps = []
    for core in range(N_CORES):
        m = dict(prep)
        m["x0"] = np.ascontiguousarray(
            im[core * IMGS_PER_CORE : (core + 1) * IMGS_PER_CORE])
        in_maps.append(m)
    res = run_bass_kernel_spmd(nc, in_maps, list(range(N_CORES))).results
    return np.concatenate([r["out"] for r in res], axis=0)


# revision 26
# speedup vs baseline: 1.6911x; 1.4834x over previous
"""BinaryVGG16 forward on 8 TRN2 NeuronCores, data-parallel (2 images/core).

Strategy
--------
- Layer 0 (fp32 conv 3->64): host-side im2col to [27, H*W]; single true-fp32
  PE matmul per row-pair tile; fused BN+sign via ScalarE Sign activation with
  per-channel scale/bias -> fp8 +-1 activations in a zero-padded layout.
- Layers 1..12 (binary convs): weights sign(w) in fp8, activations +-1 fp8.
  Conv = 9 shifted matmuls (one per 3x3 tap) accumulating in PSUM; channel
  blocks of 128 on partitions; optional fp8 DoubleRow pairs two 128-blocks
  per instruction. PSUM holds exact integer counts S. BN+binarize folds to
  sign(S*A + B) with A = bn_scale*alpha, B = bn_shift (fp32, host-folded),
  applied by one ScalarE pass (PSUM -> fp8 SBUF, padded layout).
- Maxpool commutes with sign(A*x+B) for A>=0, so pooling runs on the fp32
  PSUM values (ScalarE copy + two VectorE max ops) before the sign pass.
- Head: ScalarE Sign accum_out gives per-channel sums over the 7x7 map
  (= 49*mean); FC as 4 accumulated K=128 fp32 matmuls; bias+1/49 via ScalarE.

The final logits match the jax reference bitwise in emulation; on-device
differences are limited to fp32 rounding in layer 0 and the FC head.
"""

import sys

for _p in ("/opt/trn_rl_repo", "/root/.axon_site/_ro/trn_rl_repo"):
    if _p not in sys.path:
        sys.path.append(_p)

import numpy as np
from concourse import bacc, tile, mybir
from concourse.bass_utils import run_bass_kernel_spmd

F8 = mybir.dt.float8e4
F32 = mybir.dt.float32
DR = mybir.MatmulPerfMode.DoubleRow

USE_DR = True   # fp8 DoubleRow for channel-block pairs
USE_DUP = True  # dup-format activations for 64-ch layers (L1/L2) + colpack L1

N_CORES = 8
IMGS_PER_CORE = 2
CFG = [64, 64, 128, 128, 256, 256, 256, 512, 512, 512, 512, 512, 512]
POOL_AFTER = {1, 3, 6, 9, 12}
NUM_CLASSES = 10

# per-layer tiling: rows of conv output computed per PSUM tile
ROWS_PER_TILE = {1: 2, 2: 4, 3: 4, 4: 8, 5: 8, 6: 8, 7: 16, 8: 16, 9: 16,
                 10: 14, 11: 14, 12: 14}


def _align16(x):
    return (x + 15) & ~15


class LayerCfg:
    def __init__(self, i):
        self.i = i
        self.c_in = 3 if i == 0 else CFG[i - 1]
        self.c_out = CFG[i]
        # input spatial (= conv output spatial)
        h = 224
        for j in range(i):
            if j in POOL_AFTER:
                h //= 2
        self.h = h
        self.pw = h + 2
        self.s = (h + 2) * self.pw
        self.bs = _align16(self.s + 32)  # block stride, data at +16
        self.kg = max(1, self.c_in // 128)
        self.p_in = min(self.c_in, 128)
        self.mt = (self.c_out + 127) // 128
        self.pooled = i in POOL_AFTER
        self.h_out = h // 2 if self.pooled else h
        self.pw_out = self.h_out + 2
        self.bs_out = _align16((self.h_out + 2) * self.pw_out + 32)


LC = [LayerCfg(i) for i in range(13)]


# ---------------------------------------------------------------------------
# host-side parameter preparation
# ---------------------------------------------------------------------------

def _prep_params(params):
    f8np = mybir.dt.np(F8)
    out = {}
    for i in range(13):
        w = np.asarray(params["convs"][i], np.float32)
        g, b, m, v = [np.asarray(t, np.float32) for t in params["bns"][i]]
        scale = g / np.sqrt(v + np.float32(1e-5))
        shift = b - m * scale
        if i == 0:
            # lhsT [27, M]: k = c*9 + dy*3 + dx; M duplicated when dup format
            lhsT = w.transpose(1, 2, 3, 0).reshape(27, 64).astype(np.float32)
            if USE_DUP:
                lhsT = np.concatenate([lhsT, lhsT], axis=1)  # [27, 128]
            out["w0"] = np.ascontiguousarray(lhsT)
            ab = np.stack([scale, shift])
            out["ab0"] = np.concatenate([ab, ab], 1) if USE_DUP else ab
        else:
            lc = LC[i]
            alpha = np.mean(np.abs(w), axis=(1, 2, 3), dtype=np.float32)
            A = scale * np.minimum(alpha, np.float32(1.0))
            assert np.all(A >= 0), "pool/sign commute needs A >= 0"
            ab = np.stack([A, shift])
            if USE_DUP and i == 1:
                ab = np.concatenate([ab, ab], 1)  # replicated on dup half
            out[f"ab{i}"] = ab
            ws = np.sign(w).astype(np.float32)  # [O, C, 3, 3]
            C, O = lc.c_in, lc.c_out
            if USE_DUP and i in (1, 2):
                # dup-format lhsT [128, 6, 1, O]: slots 0-2 pair (dy, dx=-1)
                # on rows 0:64 with (dy, dx=0) on rows 64:128 (the shifted
                # dup half); slots 3-5 are the dx=+1 singles (upper rows 0).
                wp = np.zeros((128, 6, 1, O), np.float32)
                for s in range(3):
                    wp[0:64, s, 0] = ws[:, :, s, 0].T  # dx=-1, dy=s-1
                    wp[64:128, s, 0] = ws[:, :, s, 1].T  # dx=0
                    wp[0:64, s + 3, 0] = ws[:, :, s, 2].T  # dx=+1
            else:
                wst = ws.transpose(1, 2, 3, 0).reshape(C, 9, O)  # [C, t, O]
                if lc.c_in >= 128:
                    wp = wst.reshape(lc.kg, 128, 9, O).transpose(1, 2, 0, 3)
                else:
                    wp = wst.reshape(1, C, 9, O).transpose(1, 2, 0, 3)
            # [P, 9|6, KG, O]
            out[f"w{i}"] = np.ascontiguousarray(wp).astype(f8np)
    fc_w = np.asarray(params["fc_w"], np.float32)  # [10, 512]
    fc_b = np.asarray(params["fc_b"], np.float32)
    out["fcw"] = np.ascontiguousarray(
        fc_w.T.reshape(4, 128, NUM_CLASSES).transpose(1, 0, 2)
    )  # [128, 4, 10]
    out["fcb"] = fc_b
    return out


def _im2col(x):
    n = x.shape[0]
    xp = np.zeros((n, 3, 226, 226), np.float32)
    xp[:, :, 1:225, 1:225] = x
    im = np.empty((n, 27, 224 * 224), np.float32)
    for c in range(3):
        for dy in range(3):
            for dx in range(3):
                im[:, c * 9 + dy * 3 + dx] = xp[
                    :, c, dy : dy + 224, dx : dx + 224
                ].reshape(n, -1)
    return im


# ---------------------------------------------------------------------------
# device program
# ---------------------------------------------------------------------------

def _emit_borders(nc, buf, lc_out_pw, h_out, bs, kg):
    """Zero the pad ring of one act buffer (data at offset +16)."""
    pw = lc_out_pw
    for g in range(kg):
        # front pad + row0 + row1's left col
        nc.vector.memset(buf[:, g, 0 : 16 + pw + 1], 0.0)
        # (cols pw-2, pw-1 of row r, col 0 of row r+1), r = 1..h_out-1.
        # pw-2 is interior for the primary half (rewritten by the sign pass
        # later) but is the never-written tail col of shifted dup halves --
        # zeroing here keeps matmul rhs reads NaN-free.
        if h_out > 1:
            v = buf[:, g, 16 + 2 * pw - 2 : 16 + (h_out + 1) * pw - 2]
            v = v.rearrange("p (r c) -> p r c", r=h_out - 1, c=pw)
            nc.vector.memset(v[:, :, 0:3], 0.0)
        # last row's cols pw-2, pw-1 + bottom row + tail pad
        nc.vector.memset(buf[:, g, 16 + (h_out + 2) * pw - pw - 2 : bs], 0.0)


def build_nc():
    nc = bacc.Bacc("TRN2")
    d_x0 = nc.declare_dram_parameter(
        "x0", [IMGS_PER_CORE, 27, 224 * 224], F32, isOutput=False
    )
    m0 = 128 if USE_DUP else 64
    d_w = {0: nc.declare_dram_parameter("w0", [27, m0], F32, isOutput=False)}
    d_ab = {}
    for i in range(13):
        lc = LC[i]
        if i > 0:
            if USE_DUP and i in (1, 2):
                d_w[i] = nc.declare_dram_parameter(
                    f"w{i}", [128, 6, 1, lc.c_out], F8, isOutput=False)
            else:
                d_w[i] = nc.declare_dram_parameter(
                    f"w{i}", [lc.p_in, 9, lc.kg, lc.c_out], F8, isOutput=False)
        nab = 2 * lc.c_out if (USE_DUP and i in (0, 1)) else lc.c_out
        d_ab[i] = nc.declare_dram_parameter(
            f"ab{i}", [2, nab], F32, isOutput=False
        )
    d_fcw = nc.declare_dram_parameter("fcw", [128, 4, NUM_CLASSES], F32,
                                      isOutput=False)
    d_fcb = nc.declare_dram_parameter("fcb", [NUM_CLASSES], F32, isOutput=False)
    d_out = nc.declare_dram_parameter("out", [IMGS_PER_CORE, NUM_CLASSES], F32,
                                      isOutput=True)

    with tile.TileContext(nc) as tc:
        with (
            tc.tile_pool(name="wpool", bufs=2) as wpool,
            tc.tile_pool(name="abpool", bufs=1) as abpool,
            tc.tile_pool(name="iopool", bufs=4) as iopool,
            tc.tile_pool(name="tmppool", bufs=4) as tmppool,
            tc.tile_pool(name="pspool", bufs=8, space="PSUM") as pspool,
        ):
            # per-channel A/B columns, persistent (tiny)
            ab_sb = {}
            for i in range(13):
                lc = LC[i]
                nch = 2 * lc.c_out if (USE_DUP and i in (0, 1)) else lc.c_out
                t = abpool.tile([min(128, nch), 2 * lc.mt], F32,
                                name=f"ab_sb{i}")
                for m in range(lc.mt):
                    sl = slice(128 * m, 128 * m + min(128, nch - 128 * m))
                    nc.sync.dma_start(out=t[:, 2 * m], in_=d_ab[i][0, sl])
                    nc.sync.dma_start(out=t[:, 2 * m + 1], in_=d_ab[i][1, sl])
                ab_sb[i] = t
            fcw_sb = abpool.tile([128, 4, NUM_CLASSES], F32, name="fcw_sb")
            nc.sync.dma_start(out=fcw_sb[:], in_=d_fcw[:])
            fcb_sb = abpool.tile([NUM_CLASSES, 1], F32, name="fcb_sb")
            nc.sync.dma_start(out=fcb_sb[:, 0], in_=d_fcb[:])
            w0_sb = abpool.tile([27, m0], F32, name="w0_sb")
            nc.sync.dma_start(out=w0_sb[:], in_=d_w[0][:])

            for img in range(IMGS_PER_CORE):
                with tc.tile_pool(name=f"acts{img}", bufs=1) as acts:
                    bufs = {}
                    for i in range(13):
                        lc = LC[i]
                        if i < 12:
                            nxt = LC[i + 1]
                            p = 128 if (USE_DUP and i in (0, 1)) else nxt.p_in
                            bufs[i] = acts.tile(
                                [p, nxt.kg, nxt.bs], F8,
                                name=f"act{img}_{i}",
                            )
                            _emit_borders(nc, bufs[i], nxt.pw, nxt.h, nxt.bs,
                                          nxt.kg)

                    # ---- layer 0: fp32 im2col conv + BN + sign
                    lc = LC[0]
                    out_buf = bufs[0]
                    for r0 in range(0, 224, 2):
                        rhs = iopool.tile([27, 448], F32, tag="l0rhs", bufs=3)
                        nc.sync.dma_start(
                            out=rhs[:],
                            in_=d_x0[img, :, r0 * 224 : (r0 + 2) * 224],
                        )
                        ps = pspool.tile([m0, 512], F32, tag="ps", bufs=8)
                        nc.tensor.matmul(ps[:, 0:448], w0_sb[:], rhs[:],
                                         start=True, stop=True)
                        st = 16 + (r0 + 1) * 226 + 1
                        dst = out_buf[0:64, 0, st : st + 2 * 226]
                        dst = dst.rearrange("p (r c) -> p r c", r=2, c=226)
                        nc.scalar.activation(
                            dst[:, :, 0:224],
                            ps[0:64, 0:448].rearrange("p (r c) -> p r c", r=2,
                                                      c=224),
                            mybir.ActivationFunctionType.Sign,
                            bias=ab_sb[0][0:64, 1:2],
                            scale=ab_sb[0][0:64, 0:1],
                        )
                        if USE_DUP:
                            # dup half: same values, shifted left by one,
                            # on partitions 64:128 (channels duplicated in w0)
                            dst2 = out_buf[64:128, 0, st - 1 : st - 1 + 452]
                            dst2 = dst2.rearrange("p (r c) -> p r c", r=2,
                                                  c=226)
                            nc.scalar.activation(
                                dst2[:, :, 0:224],
                                ps[64:128, 0:448].rearrange(
                                    "p (r c) -> p r c", r=2, c=224),
                                mybir.ActivationFunctionType.Sign,
                                bias=ab_sb[0][64:128, 1:2],
                                scale=ab_sb[0][64:128, 0:1],
                            )

                    # ---- layers 1..12: binary convs
                    for i in range(1, 13):
                        lc = LC[i]
                        dup_in = USE_DUP and i in (1, 2)
                        nslots = 6 if dup_in else 9
                        wp_part = 128 if dup_in else lc.p_in
                        w_t = wpool.tile([wp_part, nslots, lc.kg, lc.c_out],
                                         F8, tag="w", name=f"w_sb{img}_{i}")
                        nc.sync.dma_start(out=w_t[:], in_=d_w[i][:])
                        in_buf = bufs[i - 1]
                        out_buf = bufs[i] if i < 12 else None
                        feat = None
                        if i == 12:
                            feat = tmppool.tile([128, 4], F32, tag="feat",
                                                name=f"feat{img}")
                        if dup_in:
                            taps = [(s, (s - 1) * lc.pw - 1) for s in range(3)]
                            taps += [(s, (s - 4) * lc.pw + 1)
                                     for s in range(3, 6)]
                        else:
                            taps = [((dy + 1) * 3 + dx + 1, dy * lc.pw + dx)
                                    for dy in (-1, 0, 1) for dx in (-1, 0, 1)]

                        def conv_mms(ps_ap, f0, n, msl, _lc=lc, _w=w_t,
                                     _in=in_buf, _taps=taps):
                            nmm = 0
                            tot = len(_taps) * _lc.kg
                            for (slot, off) in _taps:
                                base = 16 + f0 + off
                                kg = 0
                                while kg < _lc.kg:
                                    if USE_DR and _lc.kg - kg >= 2:
                                        nc.tensor.matmul(
                                            ps_ap,
                                            _w[:, slot, kg : kg + 2, msl],
                                            _in[:, kg : kg + 2,
                                                base : base + n],
                                            start=(nmm == 0),
                                            stop=(nmm + 2 == tot),
                                            perf_mode=DR)
                                        nmm += 2
                                        kg += 2
                                    else:
                                        nc.tensor.matmul(
                                            ps_ap,
                                            _w[:, slot, kg, msl],
                                            _in[:, kg, base : base + n],
                                            start=(nmm == 0),
                                            stop=(nmm + 1 == tot))
                                        nmm += 1
                                        kg += 1

                        if USE_DUP and i == 1:
                            # column-packed: two row-pair tiles per PSUM tile
                            # (A -> psum[0:64] = odd pooled rows -> primary,
                            #  B -> psum[64:128] = even pooled rows -> dup)
                            pw2 = lc.pw_out
                            w2 = lc.h // 2
                            for tp in range(lc.h // 4):
                                rA = 4 * tp
                                rB = rA + 2
                                ps = pspool.tile([128, 512], F32, tag="ps",
                                                 bufs=8)
                                even = tmppool.tile([128, lc.pw], F32,
                                                    tag="poolA", bufs=3)
                                rmax = tmppool.tile([128, lc.pw], F32,
                                                    tag="poolB", bufs=3)
                                cmax = tmppool.tile([128, w2], F32,
                                                    tag="poolC", bufs=3)
                                for hx, rh in ((0, rA), (1, rB)):
                                    sl = slice(64 * hx, 64 * hx + 64)
                                    conv_mms(ps[sl, 0 : 2 * lc.pw],
                                             (rh + 1) * lc.pw, 2 * lc.pw,
                                             slice(0, 64))
                                    nc.scalar.copy(even[sl, :],
                                                   ps[sl, 0 : lc.pw])
                                    nc.vector.tensor_tensor(
                                        rmax[sl, :], even[sl, :],
                                        ps[sl, lc.pw : 2 * lc.pw],
                                        mybir.AluOpType.max)
                                    nc.vector.tensor_tensor(
                                        cmax[sl, :],
                                        rmax[sl, 1 : 1 + 2 * w2 : 2],
                                        rmax[sl, 2 : 2 + 2 * w2 : 2],
                                        mybir.AluOpType.max)
                                pA = rA // 2 + 1
                                stA = 16 + pA * pw2 + 1
                                nc.scalar.activation(
                                    out_buf[0:64, 0, stA : stA + w2],
                                    cmax[0:64, :],
                                    mybir.ActivationFunctionType.Sign,
                                    bias=ab_sb[1][0:64, 1:2],
                                    scale=ab_sb[1][0:64, 0:1])
                                stB = 16 + (pA + 1) * pw2
                                nc.scalar.activation(
                                    out_buf[64:128, 0, stB : stB + w2],
                                    cmax[64:128, :],
                                    mybir.ActivationFunctionType.Sign,
                                    bias=ab_sb[1][64:128, 1:2],
                                    scale=ab_sb[1][64:128, 0:1])
                            # complement copies to finish the dup format
                            nrow = w2 // 2
                            # primary even rows <- dup even rows (shift -1)
                            st_d = 16 + 2 * pw2
                            v_src = out_buf[64:128, 0,
                                            st_d : st_d + nrow * 2 * pw2]
                            v_src = v_src.rearrange("p (r c) -> p r c",
                                                    r=nrow, c=2 * pw2)
                            v_dst = out_buf[0:64, 0,
                                            st_d + 1 : st_d + 1
                                            + nrow * 2 * pw2]
                            v_dst = v_dst.rearrange("p (r c) -> p r c",
                                                    r=nrow, c=2 * pw2)
                            nc.sync.dma_start(out=v_dst[:, :, 0:w2],
                                              in_=v_src[:, :, 0:w2])
                            # dup odd rows <- primary odd rows (shift +1)
                            st_p = 16 + pw2
                            u_src = out_buf[0:64, 0,
                                            st_p + 1 : st_p + 1
                                            + nrow * 2 * pw2]
                            u_src = u_src.rearrange("p (r c) -> p r c",
                                                    r=nrow, c=2 * pw2)
                            u_dst = out_buf[64:128, 0,
                                            st_p : st_p + nrow * 2 * pw2]
                            u_dst = u_dst.rearrange("p (r c) -> p r c",
                                                    r=nrow, c=2 * pw2)
                            nc.sync.dma_start(out=u_dst[:, :, 0:w2],
                                              in_=u_src[:, :, 0:w2])
                            continue

                        rpt = ROWS_PER_TILE[i]
                        r0 = 0
                        while r0 < lc.h:
                            rr = min(rpt, lc.h - r0)
                            f0 = (r0 + 1) * lc.pw
                            n = rr * lc.pw
                            for m in range(lc.mt):
                                mw = min(128, lc.c_out - 128 * m)
                                msl = slice(128 * m, 128 * m + mw)
                                ps = pspool.tile([mw, 512], F32, tag="ps",
                                                 bufs=8)
                                conv_mms(ps[:, 0:n], f0, n, msl)
                                if lc.pooled:
                                    # 2x2 maxpool on fp32 PSUM, then sign
                                    half = rr // 2
                                    pse = ps[:, 0:n].rearrange(
                                        "p (r c) -> p r c", r=rr, c=lc.pw)
                                    even = tmppool.tile(
                                        [mw, half * lc.pw], F32, tag="poolA",
                                        bufs=3)
                                    nc.scalar.copy(
                                        even[:].rearrange(
                                            "p (r c) -> p r c", r=half,
                                            c=lc.pw),
                                        pse[:, 0 : rr : 2, :])
                                    rmax = tmppool.tile(
                                        [mw, half * lc.pw], F32, tag="poolB",
                                        bufs=3)
                                    nc.vector.tensor_tensor(
                                        rmax[:].rearrange(
                                            "p (r c) -> p r c", r=half,
                                            c=lc.pw),
                                        even[:].rearrange(
                                            "p (r c) -> p r c", r=half,
                                            c=lc.pw),
                                        pse[:, 1 : rr : 2, :],
                                        mybir.AluOpType.max)
                                    rm3 = rmax[:].rearrange(
                                        "p (r c) -> p r c", r=half, c=lc.pw)
                                    w2 = lc.h // 2
                                    cmax = tmppool.tile(
                                        [mw, half * w2], F32, tag="poolC",
                                        bufs=3)
                                    nc.vector.tensor_tensor(
                                        cmax[:].rearrange(
                                            "p (r c) -> p r c", r=half, c=w2),
                                        rm3[:, :, 1 : 1 + 2 * w2 : 2],
                                        rm3[:, :, 2 : 2 + 2 * w2 : 2],
                                        mybir.AluOpType.max)
                                    if i == 12:
                                        scratch = tmppool.tile(
                                            [mw, 49], F32, tag="sc12", bufs=2)
                                        nc.scalar.activation(
                                            scratch[:], cmax[:],
                                            mybir.ActivationFunctionType.Sign,
                                            bias=ab_sb[i][:, 2 * m + 1 :
                                                          2 * m + 2],
                                            scale=ab_sb[i][:, 2 * m :
                                                           2 * m + 1],
                                            accum_out=feat[:, m : m + 1])
                                    else:
                                        pw2 = lc.pw_out
                                        st = 16 + (r0 // 2 + 1) * pw2 + 1
                                        dst = out_buf[:, m,
                                                      st : st + half * pw2]
                                        nc.scalar.activation(
                                            dst.rearrange(
                                                "p (r c) -> p r c", r=half,
                                                c=pw2)[:, :, 0:w2],
                                            cmax[:].rearrange(
                                                "p (r c) -> p r c", r=half,
                                                c=w2),
                                            mybir.ActivationFunctionType.Sign,
                                            bias=ab_sb[i][:, 2 * m + 1 :
                                                          2 * m + 2],
                                            scale=ab_sb[i][:, 2 * m :
                                                           2 * m + 1])
                                else:
                                    # interior-only write (borders stay zero)
                                    st = 16 + f0 + 1
                                    dst = out_buf[:, m, st : st + rr * lc.pw]
                                    dst = dst.rearrange(
                                        "p (r c) -> p r c", r=rr, c=lc.pw)
                                    src = ps[:, 0:n].rearrange(
                                        "p (r c) -> p r c", r=rr, c=lc.pw)
                                    nc.scalar.activation(
                                        dst[:, :, 0 : lc.h],
                                        src[:, :, 1 : 1 + lc.h],
                                        mybir.ActivationFunctionType.Sign,
                                        bias=ab_sb[i][:, 2 * m + 1 : 2 * m + 2],
                                        scale=ab_sb[i][:, 2 * m : 2 * m + 1])
                            r0 += rr

                    # ---- head: FC on accumulated sign sums (feat = 49*mean)
                    ps_fc = pspool.tile([NUM_CLASSES, 512], F32, tag="ps",
                                        bufs=8)
                    for kg in range(4):
                        nc.tensor.matmul(
                            ps_fc[:, 0:1], fcw_sb[:, kg, :],
                            feat[:, kg : kg + 1],
                            start=(kg == 0), stop=(kg == 3))
                    logits = tmppool.tile([NUM_CLASSES, 1], F32, tag="logits",
                                          bufs=2)
                    nc.scalar.activation(
                        logits[:], ps_fc[:, 0:1],
                        mybir.ActivationFunctionType.Identity,
                        bias=fcb_sb[:], scale=float(np.float32(1.0 / 49.0)))
                    nc.sync.dma_start(out=d_out[img, :], in_=logits[:, 0])

    nc.finalize()
    return nc


_NC_CACHE = None


def kernel(x, params):
    global _NC_CACHE
    x = np.asarray(x, np.float32)
    prep = _prep_params(params)
    im = _im2col(x)
    if _NC_CACHE is None:
        _NC_CACHE = build_nc()
    nc = _NC_CACHE
    in_maps = []
    for core in range(N_CORES):
        m = dict(prep)
        m["x0"] = np.ascontiguousarray(
            im[core * IMGS_PER_CORE : (core + 1) * IMGS_PER_CORE])
        in_maps.append(m)
    res = run_bass_kernel_spmd(nc, in_maps, list(range(N_CORES))).results
    return np.concatenate([r["out"] for r in res], axis=0)


# revision 28
# speedup vs baseline: 1.7897x; 1.0583x over previous
"""BinaryVGG16 forward on 8 TRN2 NeuronCores, data-parallel (2 images/core).

Strategy
--------
- Layer 0 (fp32 conv 3->64): host-side im2col to [27, H*W]; single true-fp32
  PE matmul per row-pair tile; fused BN+sign via ScalarE Sign activation with
  per-channel scale/bias -> fp8 +-1 activations in a zero-padded layout.
- Layers 1..12 (binary convs): weights sign(w) in fp8, activations +-1 fp8.
  Conv = 9 shifted matmuls (one per 3x3 tap) accumulating in PSUM; channel
  blocks of 128 on partitions; optional fp8 DoubleRow pairs two 128-blocks
  per instruction. PSUM holds exact integer counts S. BN+binarize folds to
  sign(S*A + B) with A = bn_scale*alpha, B = bn_shift (fp32, host-folded),
  applied by one ScalarE pass (PSUM -> fp8 SBUF, padded layout).
- Maxpool commutes with sign(A*x+B) for A>=0, so pooling runs on the fp32
  PSUM values (ScalarE copy + two VectorE max ops) before the sign pass.
- Head: ScalarE Sign accum_out gives per-channel sums over the 7x7 map
  (= 49*mean); FC as 4 accumulated K=128 fp32 matmuls; bias+1/49 via ScalarE.

The final logits match the jax reference bitwise in emulation; on-device
differences are limited to fp32 rounding in layer 0 and the FC head.
"""

import sys

for _p in ("/opt/trn_rl_repo", "/root/.axon_site/_ro/trn_rl_repo"):
    if _p not in sys.path:
        sys.path.append(_p)

import numpy as np
from concourse import bacc, tile, mybir
from concourse.bass_utils import run_bass_kernel_spmd

F8 = mybir.dt.float8e4
F32 = mybir.dt.float32
DR = mybir.MatmulPerfMode.DoubleRow

USE_DR = True   # fp8 DoubleRow for channel-block pairs
USE_DUP = True  # dup-format activations for 64-ch layers (L1/L2) + colpack L1

N_CORES = 8
IMGS_PER_CORE = 2
CFG = [64, 64, 128, 128, 256, 256, 256, 512, 512, 512, 512, 512, 512]
POOL_AFTER = {1, 3, 6, 9, 12}
NUM_CLASSES = 10

# per-layer tiling: rows of conv output computed per PSUM tile
ROWS_PER_TILE = {1: 2, 2: 4, 3: 4, 4: 8, 5: 8, 6: 8, 7: 16, 8: 16, 9: 16,
                 10: 14, 11: 14, 12: 14}


def _align16(x):
    return (x + 15) & ~15


class LayerCfg:
    def __init__(self, i):
        self.i = i
        self.c_in = 3 if i == 0 else CFG[i - 1]
        self.c_out = CFG[i]
        # input spatial (= conv output spatial)
        h = 224
        for j in range(i):
            if j in POOL_AFTER:
                h //= 2
        self.h = h
        self.pw = h + 2
        self.s = (h + 2) * self.pw
        self.bs = _align16(self.s + 32)  # block stride, data at +16
        self.kg = max(1, self.c_in // 128)
        self.p_in = min(self.c_in, 128)
        self.mt = (self.c_out + 127) // 128
        self.pooled = i in POOL_AFTER
        self.h_out = h // 2 if self.pooled else h
        self.pw_out = self.h_out + 2
        self.bs_out = _align16((self.h_out + 2) * self.pw_out + 32)


LC = [LayerCfg(i) for i in range(13)]


# ---------------------------------------------------------------------------
# host-side parameter preparation
# ---------------------------------------------------------------------------

def _prep_params(params):
    f8np = mybir.dt.np(F8)
    out = {}
    for i in range(13):
        w = np.asarray(params["convs"][i], np.float32)
        g, b, m, v = [np.asarray(t, np.float32) for t in params["bns"][i]]
        scale = g / np.sqrt(v + np.float32(1e-5))
        shift = b - m * scale
        if i == 0:
            # lhsT [27, M]: k = c*9 + dy*3 + dx; M duplicated when dup format
            lhsT = w.transpose(1, 2, 3, 0).reshape(27, 64).astype(np.float32)
            if USE_DUP:
                lhsT = np.concatenate([lhsT, lhsT], axis=1)  # [27, 128]
            out["w0"] = np.ascontiguousarray(lhsT)
            ab = np.stack([scale, shift])
            out["ab0"] = np.concatenate([ab, ab], 1) if USE_DUP else ab
        else:
            lc = LC[i]
            alpha = np.mean(np.abs(w), axis=(1, 2, 3), dtype=np.float32)
            A = scale * np.minimum(alpha, np.float32(1.0))
            assert np.all(A >= 0), "pool/sign commute needs A >= 0"
            ab = np.stack([A, shift])
            if USE_DUP and i == 1:
                ab = np.concatenate([ab, ab], 1)  # replicated on dup half
            out[f"ab{i}"] = ab
            ws = np.sign(w).astype(np.float32)  # [O, C, 3, 3]
            C, O = lc.c_in, lc.c_out
            if USE_DUP and i in (1, 2):
                # dup-format lhsT [128, 6, 1, O]: slots 0-2 pair (dy, dx=-1)
                # on rows 0:64 with (dy, dx=0) on rows 64:128 (the shifted
                # dup half); slots 3-5 are the dx=+1 singles (upper rows 0).
                wp = np.zeros((128, 6, 1, O), np.float32)
                for s in range(3):
                    wp[0:64, s, 0] = ws[:, :, s, 0].T  # dx=-1, dy=s-1
                    wp[64:128, s, 0] = ws[:, :, s, 1].T  # dx=0
                    wp[0:64, s + 3, 0] = ws[:, :, s, 2].T  # dx=+1
            else:
                wst = ws.transpose(1, 2, 3, 0).reshape(C, 9, O)  # [C, t, O]
                if lc.c_in >= 128:
                    wp = wst.reshape(lc.kg, 128, 9, O).transpose(1, 2, 0, 3)
                else:
                    wp = wst.reshape(1, C, 9, O).transpose(1, 2, 0, 3)
            # [P, 9|6, KG, O]
            out[f"w{i}"] = np.ascontiguousarray(wp).astype(f8np)
    fc_w = np.asarray(params["fc_w"], np.float32)  # [10, 512]
    fc_b = np.asarray(params["fc_b"], np.float32)
    out["fcw"] = np.ascontiguousarray(
        fc_w.T.reshape(4, 128, NUM_CLASSES).transpose(1, 0, 2)
    )  # [128, 4, 10]
    out["fcb"] = fc_b
    return out


def _im2col(x):
    n = x.shape[0]
    xp = np.zeros((n, 3, 226, 226), np.float32)
    xp[:, :, 1:225, 1:225] = x
    im = np.empty((n, 27, 224 * 224), np.float32)
    for c in range(3):
        for dy in range(3):
            for dx in range(3):
                im[:, c * 9 + dy * 3 + dx] = xp[
                    :, c, dy : dy + 224, dx : dx + 224
                ].reshape(n, -1)
    return im


# ---------------------------------------------------------------------------
# device program
# ---------------------------------------------------------------------------

def _emit_borders(nc, buf, lc_out_pw, h_out, bs, kg):
    """Zero the pad ring of one act buffer (data at offset +16)."""
    pw = lc_out_pw
    for g in range(kg):
        # front pad + row0 + row1's left col
        nc.vector.memset(buf[:, g, 0 : 16 + pw + 1], 0.0)
        # (cols pw-2, pw-1 of row r, col 0 of row r+1), r = 1..h_out-1.
        # pw-2 is interior for the primary half (rewritten by the sign pass
        # later) but is the never-written tail col of shifted dup halves --
        # zeroing here keeps matmul rhs reads NaN-free.
        if h_out > 1:
            v = buf[:, g, 16 + 2 * pw - 2 : 16 + (h_out + 1) * pw - 2]
            v = v.rearrange("p (r c) -> p r c", r=h_out - 1, c=pw)
            nc.vector.memset(v[:, :, 0:3], 0.0)
        # last row's cols pw-2, pw-1 + bottom row + tail pad
        nc.vector.memset(buf[:, g, 16 + (h_out + 2) * pw - pw - 2 : bs], 0.0)


def build_nc():
    nc = bacc.Bacc("TRN2")
    d_x0 = nc.declare_dram_parameter(
        "x0", [IMGS_PER_CORE, 27, 224 * 224], F32, isOutput=False
    )
    m0 = 128 if USE_DUP else 64
    d_w = {0: nc.declare_dram_parameter("w0", [27, m0], F32, isOutput=False)}
    d_ab = {}
    for i in range(13):
        lc = LC[i]
        if i > 0:
            if USE_DUP and i in (1, 2):
                d_w[i] = nc.declare_dram_parameter(
                    f"w{i}", [128, 6, 1, lc.c_out], F8, isOutput=False)
            else:
                d_w[i] = nc.declare_dram_parameter(
                    f"w{i}", [lc.p_in, 9, lc.kg, lc.c_out], F8, isOutput=False)
        nab = 2 * lc.c_out if (USE_DUP and i in (0, 1)) else lc.c_out
        d_ab[i] = nc.declare_dram_parameter(
            f"ab{i}", [2, nab], F32, isOutput=False
        )
    d_fcw = nc.declare_dram_parameter("fcw", [128, 4, NUM_CLASSES], F32,
                                      isOutput=False)
    d_fcb = nc.declare_dram_parameter("fcb", [NUM_CLASSES], F32, isOutput=False)
    d_out = nc.declare_dram_parameter("out", [IMGS_PER_CORE, NUM_CLASSES], F32,
                                      isOutput=True)

    with tile.TileContext(nc) as tc:
        with (
            tc.tile_pool(name="wpool", bufs=2) as wpool,
            tc.tile_pool(name="abpool", bufs=1) as abpool,
            tc.tile_pool(name="iopool", bufs=4) as iopool,
            tc.tile_pool(name="tmppool", bufs=4) as tmppool,
            tc.tile_pool(name="pspool", bufs=8, space="PSUM") as pspool,
        ):
            # per-channel A/B columns, persistent (tiny)
            ab_sb = {}
            for i in range(13):
                lc = LC[i]
                nch = 2 * lc.c_out if (USE_DUP and i in (0, 1)) else lc.c_out
                t = abpool.tile([min(128, nch), 2 * lc.mt], F32,
                                name=f"ab_sb{i}")
                for m in range(lc.mt):
                    sl = slice(128 * m, 128 * m + min(128, nch - 128 * m))
                    nc.sync.dma_start(out=t[:, 2 * m], in_=d_ab[i][0, sl])
                    nc.sync.dma_start(out=t[:, 2 * m + 1], in_=d_ab[i][1, sl])
                ab_sb[i] = t
            fcw_sb = abpool.tile([128, 4, NUM_CLASSES], F32, name="fcw_sb")
            nc.sync.dma_start(out=fcw_sb[:], in_=d_fcw[:])
            fcb_sb = abpool.tile([NUM_CLASSES, 1], F32, name="fcb_sb")
            nc.sync.dma_start(out=fcb_sb[:, 0], in_=d_fcb[:])
            w0_sb = abpool.tile([27, m0], F32, name="w0_sb")
            nc.sync.dma_start(out=w0_sb[:], in_=d_w[0][:])

            for img in range(IMGS_PER_CORE):
                with tc.tile_pool(name=f"acts{img}", bufs=1) as acts:
                    bufs = {}
                    for i in range(13):
                        lc = LC[i]
                        if i < 12:
                            nxt = LC[i + 1]
                            p = 128 if (USE_DUP and i in (0, 1)) else nxt.p_in
                            bufs[i] = acts.tile(
                                [p, nxt.kg, nxt.bs], F8,
                                name=f"act{img}_{i}",
                            )
                            _emit_borders(nc, bufs[i], nxt.pw, nxt.h, nxt.bs,
                                          nxt.kg)

                    # ---- layer 0: fp32 im2col conv + BN + sign
                    lc = LC[0]
                    out_buf = bufs[0]
                    for r0 in range(0, 224, 2):
                        rhs = iopool.tile([27, 448], F32, tag="l0rhs", bufs=3)
                        nc.sync.dma_start(
                            out=rhs[:],
                            in_=d_x0[img, :, r0 * 224 : (r0 + 2) * 224],
                        )
                        ps = pspool.tile([m0, 512], F32, tag="ps", bufs=8)
                        nc.tensor.matmul(ps[:, 0:448], w0_sb[:], rhs[:],
                                         start=True, stop=True)
                        st = 16 + (r0 + 1) * 226 + 1
                        dst = out_buf[0:64, 0, st : st + 2 * 226]
                        dst = dst.rearrange("p (r c) -> p r c", r=2, c=226)
                        nc.scalar.activation(
                            dst[:, :, 0:224],
                            ps[0:64, 0:448].rearrange("p (r c) -> p r c", r=2,
                                                      c=224),
                            mybir.ActivationFunctionType.Sign,
                            bias=ab_sb[0][0:64, 1:2],
                            scale=ab_sb[0][0:64, 0:1],
                        )
                        if USE_DUP:
                            # dup half: same values, shifted left by one,
                            # on partitions 64:128 (channels duplicated in w0)
                            dst2 = out_buf[64:128, 0, st - 1 : st - 1 + 452]
                            dst2 = dst2.rearrange("p (r c) -> p r c", r=2,
                                                  c=226)
                            nc.scalar.activation(
                                dst2[:, :, 0:224],
                                ps[64:128, 0:448].rearrange(
                                    "p (r c) -> p r c", r=2, c=224),
                                mybir.ActivationFunctionType.Sign,
                                bias=ab_sb[0][64:128, 1:2],
                                scale=ab_sb[0][64:128, 0:1],
                            )

                    # ---- layers 1..12: binary convs
                    for i in range(1, 13):
                        lc = LC[i]
                        dup_in = USE_DUP and i in (1, 2)
                        nslots = 6 if dup_in else 9
                        wp_part = 128 if dup_in else lc.p_in
                        w_t = wpool.tile([wp_part, nslots, lc.kg, lc.c_out],
                                         F8, tag="w", name=f"w_sb{img}_{i}")
                        nc.sync.dma_start(out=w_t[:], in_=d_w[i][:])
                        in_buf = bufs[i - 1]
                        out_buf = bufs[i] if i < 12 else None
                        feat = None
                        if i == 12:
                            feat = tmppool.tile([128, 4], F32, tag="feat",
                                                name=f"feat{img}")
                        if dup_in:
                            taps = [(s, (s - 1) * lc.pw - 1) for s in range(3)]
                            taps += [(s, (s - 4) * lc.pw + 1)
                                     for s in range(3, 6)]
                        else:
                            taps = [((dy + 1) * 3 + dx + 1, dy * lc.pw + dx)
                                    for dy in (-1, 0, 1) for dx in (-1, 0, 1)]

                        def conv_mms(ps_ap, f0, n, msl, _lc=lc, _w=w_t,
                                     _in=in_buf, _taps=taps):
                            nmm = 0
                            tot = len(_taps) * _lc.kg
                            for (slot, off) in _taps:
                                base = 16 + f0 + off
                                kg = 0
                                while kg < _lc.kg:
                                    if USE_DR and _lc.kg - kg >= 2:
                                        nc.tensor.matmul(
                                            ps_ap,
                                            _w[:, slot, kg : kg + 2, msl],
                                            _in[:, kg : kg + 2,
                                                base : base + n],
                                            start=(nmm == 0),
                                            stop=(nmm + 2 == tot),
                                            perf_mode=DR)
                                        nmm += 2
                                        kg += 2
                                    else:
                                        nc.tensor.matmul(
                                            ps_ap,
                                            _w[:, slot, kg, msl],
                                            _in[:, kg, base : base + n],
                                            start=(nmm == 0),
                                            stop=(nmm + 1 == tot))
                                        nmm += 1
                                        kg += 1

                        if USE_DUP and i == 1:
                            # column-packed: two row-pair tiles per PSUM tile
                            # (A -> psum[0:64] = odd pooled rows -> primary,
                            #  B -> psum[64:128] = even pooled rows -> dup)
                            pw2 = lc.pw_out
                            w2 = lc.h // 2
                            npair = 2 * lc.pw
                            for tp in range(lc.h // 4):
                                rA = 4 * tp
                                rB = rA + 2
                                ps = pspool.tile([128, 512], F32, tag="ps",
                                                 bufs=8)
                                # interleave A/B matmuls so consecutive MMs
                                # hit different col-groups (subarray overlap)
                                for si, (slot, off) in enumerate(taps):
                                    for hx, rh in ((0, rA), (1, rB)):
                                        sl = slice(64 * hx, 64 * hx + 64)
                                        base = 16 + (rh + 1) * lc.pw + off
                                        nc.tensor.matmul(
                                            ps[sl, 0:npair],
                                            w_t[:, slot, 0, 0:64],
                                            in_buf[:, 0, base : base + npair],
                                            start=(si == 0),
                                            stop=(si == len(taps) - 1))
                                # sign the whole tile at once, then pool +-1s
                                sgn = tmppool.tile([128, npair], F32,
                                                   tag="poolS", bufs=3)
                                nc.scalar.activation(
                                    sgn[:], ps[:, 0:npair],
                                    mybir.ActivationFunctionType.Sign,
                                    bias=ab_sb[1][:, 1:2],
                                    scale=ab_sb[1][:, 0:1])
                                rmax = tmppool.tile([128, lc.pw], F32,
                                                    tag="poolB", bufs=3)
                                nc.vector.tensor_tensor(
                                    rmax[:], sgn[:, 0 : lc.pw],
                                    sgn[:, lc.pw : npair],
                                    mybir.AluOpType.max)
                                cmax = tmppool.tile([128, w2], F32,
                                                    tag="poolC", bufs=3)
                                nc.vector.tensor_tensor(
                                    cmax[:],
                                    rmax[:, 1 : 1 + 2 * w2 : 2],
                                    rmax[:, 2 : 2 + 2 * w2 : 2],
                                    mybir.AluOpType.max)
                                pA = rA // 2 + 1
                                stA = 16 + pA * pw2 + 1
                                nc.scalar.copy(
                                    out_buf[0:64, 0, stA : stA + w2],
                                    cmax[0:64, :])
                                stB = 16 + (pA + 1) * pw2
                                nc.vector.tensor_copy(
                                    out_buf[64:128, 0, stB : stB + w2],
                                    cmax[64:128, :])
                            # complement copies to finish the dup format
                            nrow = w2 // 2
                            # primary even rows <- dup even rows (shift -1)
                            st_d = 16 + 2 * pw2
                            v_src = out_buf[64:128, 0,
                                            st_d : st_d + nrow * 2 * pw2]
                            v_src = v_src.rearrange("p (r c) -> p r c",
                                                    r=nrow, c=2 * pw2)
                            v_dst = out_buf[0:64, 0,
                                            st_d + 1 : st_d + 1
                                            + nrow * 2 * pw2]
                            v_dst = v_dst.rearrange("p (r c) -> p r c",
                                                    r=nrow, c=2 * pw2)
                            nc.sync.dma_start(out=v_dst[:, :, 0:w2],
                                              in_=v_src[:, :, 0:w2])
                            # dup odd rows <- primary odd rows (shift +1)
                            st_p = 16 + pw2
                            u_src = out_buf[0:64, 0,
                                            st_p + 1 : st_p + 1
                                            + nrow * 2 * pw2]
                            u_src = u_src.rearrange("p (r c) -> p r c",
                                                    r=nrow, c=2 * pw2)
                            u_dst = out_buf[64:128, 0,
                                            st_p : st_p + nrow * 2 * pw2]
                            u_dst = u_dst.rearrange("p (r c) -> p r c",
                                                    r=nrow, c=2 * pw2)
                            nc.sync.dma_start(out=u_dst[:, :, 0:w2],
                                              in_=u_src[:, :, 0:w2])
                            continue

                        rpt = ROWS_PER_TILE[i]
                        r0 = 0
                        while r0 < lc.h:
                            rr = min(rpt, lc.h - r0)
                            f0 = (r0 + 1) * lc.pw
                            n = rr * lc.pw
                            for m in range(lc.mt):
                                mw = min(128, lc.c_out - 128 * m)
                                msl = slice(128 * m, 128 * m + mw)
                                ps = pspool.tile([mw, 512], F32, tag="ps",
                                                 bufs=8)
                                conv_mms(ps[:, 0:n], f0, n, msl)
                                if lc.pooled:
                                    # sign the whole psum tile, then 2x2 max
                                    # of +-1 values (matches reference order)
                                    half = rr // 2
                                    w2 = lc.h // 2
                                    sgn = tmppool.tile([mw, n], F32,
                                                       tag="poolS", bufs=3)
                                    nc.scalar.activation(
                                        sgn[:], ps[:, 0:n],
                                        mybir.ActivationFunctionType.Sign,
                                        bias=ab_sb[i][:, 2 * m + 1 :
                                                      2 * m + 2],
                                        scale=ab_sb[i][:, 2 * m : 2 * m + 1])
                                    sg3 = sgn[:].rearrange(
                                        "p (r c) -> p r c", r=rr, c=lc.pw)
                                    rmax = tmppool.tile(
                                        [mw, half * lc.pw], F32, tag="poolB",
                                        bufs=3)
                                    nc.vector.tensor_tensor(
                                        rmax[:].rearrange(
                                            "p (r c) -> p r c", r=half,
                                            c=lc.pw),
                                        sg3[:, 0 : rr : 2, :],
                                        sg3[:, 1 : rr : 2, :],
                                        mybir.AluOpType.max)
                                    rm3 = rmax[:].rearrange(
                                        "p (r c) -> p r c", r=half, c=lc.pw)
                                    if i == 12:
                                        cmax = tmppool.tile(
                                            [mw, half * w2], F32, tag="poolC",
                                            bufs=3)
                                        nc.vector.tensor_tensor(
                                            cmax[:].rearrange(
                                                "p (r c) -> p r c", r=half,
                                                c=w2),
                                            rm3[:, :, 1 : 1 + 2 * w2 : 2],
                                            rm3[:, :, 2 : 2 + 2 * w2 : 2],
                                            mybir.AluOpType.max)
                                        nc.vector.reduce_sum(
                                            feat[:, m : m + 1], cmax[:],
                                            axis=mybir.AxisListType.X)
                                    else:
                                        pw2 = lc.pw_out
                                        st = 16 + (r0 // 2 + 1) * pw2 + 1
                                        dst = out_buf[:, m,
                                                      st : st + half * pw2]
                                        nc.vector.tensor_tensor(
                                            dst.rearrange(
                                                "p (r c) -> p r c", r=half,
                                                c=pw2)[:, :, 0:w2],
                                            rm3[:, :, 1 : 1 + 2 * w2 : 2],
                                            rm3[:, :, 2 : 2 + 2 * w2 : 2],
                                            mybir.AluOpType.max)
                                else:
                                    # interior-only write (borders stay zero)
                                    st = 16 + f0 + 1
                                    dst = out_buf[:, m, st : st + rr * lc.pw]
                                    dst = dst.rearrange(
                                        "p (r c) -> p r c", r=rr, c=lc.pw)
                                    src = ps[:, 0:n].rearrange(
                                        "p (r c) -> p r c", r=rr, c=lc.pw)
                                    nc.scalar.activation(
                                        dst[:, :, 0 : lc.h],
                                        src[:, :, 1 : 1 + lc.h],
                                        mybir.ActivationFunctionType.Sign,
                                        bias=ab_sb[i][:, 2 * m + 1 : 2 * m + 2],
                                        scale=ab_sb[i][:, 2 * m : 2 * m + 1])
                            r0 += rr

                    # ---- head: FC on accumulated sign sums (feat = 49*mean)
                    ps_fc = pspool.tile([NUM_CLASSES, 512], F32, tag="ps",
                                        bufs=8)
                    for kg in range(4):
                        nc.tensor.matmul(
                            ps_fc[:, 0:1], fcw_sb[:, kg, :],
                            feat[:, kg : kg + 1],
                            start=(kg == 0), stop=(kg == 3))
                    logits = tmppool.tile([NUM_CLASSES, 1], F32, tag="logits",
                                          bufs=2)
                    nc.scalar.activation(
                        logits[:], ps_fc[:, 0:1],
                        mybir.ActivationFunctionType.Identity,
                        bias=fcb_sb[:], scale=float(np.float32(1.0 / 49.0)))
                    nc.sync.dma_start(out=d_out[img, :], in_=logits[:, 0])

    nc.finalize()
    return nc


_NC_CACHE = None


def kernel(x, params):
    global _NC_CACHE
    x = np.asarray(x, np.float32)
    prep = _prep_params(params)
    im = _im2col(x)
    if _NC_CACHE is None:
        _NC_CACHE = build_nc()
    nc = _NC_CACHE
    in_maps = []
    for core in range(N_CORES):
        m = dict(prep)
        m["x0"] = np.ascontiguousarray(
            im[core * IMGS_PER_CORE : (core + 1) * IMGS_PER_CORE])
        in_maps.append(m)
    res = run_bass_kernel_spmd(nc, in_maps, list(range(N_CORES))).results
    return np.concatenate([r["out"] for r in res], axis=0)


# revision 44
# speedup vs baseline: 1.9113x; 1.0679x over previous
"""BinaryVGG16 forward on 8 TRN2 NeuronCores, data-parallel (2 images/core).

Strategy
--------
- Layer 0 (fp32 conv 3->64): host-side im2col to [27, H*W]; single true-fp32
  PE matmul per row-pair tile; fused BN+sign via ScalarE Sign activation with
  per-channel scale/bias -> fp8 +-1 activations in a zero-padded layout.
- Layers 1..12 (binary convs): weights sign(w) in fp8, activations +-1 fp8.
  Conv = 9 shifted matmuls (one per 3x3 tap) accumulating in PSUM; channel
  blocks of 128 on partitions; optional fp8 DoubleRow pairs two 128-blocks
  per instruction. PSUM holds exact integer counts S. BN+binarize folds to
  sign(S*A + B) with A = bn_scale*alpha, B = bn_shift (fp32, host-folded),
  applied by one ScalarE pass (PSUM -> fp8 SBUF, padded layout).
- Maxpool commutes with sign(A*x+B) for A>=0, so pooling runs on the fp32
  PSUM values (ScalarE copy + two VectorE max ops) before the sign pass.
- Head: ScalarE Sign accum_out gives per-channel sums over the 7x7 map
  (= 49*mean); FC as 4 accumulated K=128 fp32 matmuls; bias+1/49 via ScalarE.

The final logits match the jax reference bitwise in emulation; on-device
differences are limited to fp32 rounding in layer 0 and the FC head.
"""

import sys

for _p in ("/opt/trn_rl_repo", "/root/.axon_site/_ro/trn_rl_repo"):
    if _p not in sys.path:
        sys.path.append(_p)

import dataclasses

import numpy as np
from concourse import bacc, tile, mybir
from concourse.bass_utils import run_bass_kernel_spmd

F8 = mybir.dt.float8e4
F32 = mybir.dt.float32
F32R = mybir.dt.float32r
DR = mybir.MatmulPerfMode.DoubleRow

USE_DR = True   # fp8 DoubleRow for channel-block pairs
USE_DUP = True  # dup-format activations for 64-ch layers (L1/L2) + colpack L1

N_CORES = 8
IMGS_PER_CORE = 2
CFG = [64, 64, 128, 128, 256, 256, 256, 512, 512, 512, 512, 512, 512]
POOL_AFTER = {1, 3, 6, 9, 12}
NUM_CLASSES = 10

# per-layer tiling: rows of conv output computed per PSUM tile
ROWS_PER_TILE = {1: 2, 2: 4, 3: 4, 4: 8, 5: 8, 6: 8, 7: 16, 8: 16, 9: 16,
                 10: 14, 11: 14, 12: 14}

import os

# layers whose input gets a +2-row-shifted duplicate block so vertical
# (dy=-1, dy=+1) tap pairs can use DoubleRow with standard block APs
TAP_PAIR = () if os.environ.get("NO_TAPPAIR") else (2, 3, 4)


def _align16(x):
    return (x + 15) & ~15


class LayerCfg:
    def __init__(self, i):
        self.i = i
        self.c_in = 3 if i == 0 else CFG[i - 1]
        self.c_out = CFG[i]
        # input spatial (= conv output spatial)
        h = 224
        for j in range(i):
            if j in POOL_AFTER:
                h //= 2
        self.h = h
        self.pw = h + 2
        self.tp = USE_DR and i in TAP_PAIR
        self.s = (h + 2) * self.pw
        self.bs = _align16(self.s + 32)  # block stride, data at +16
        self.kg = max(1, self.c_in // 128)
        self.p_in = min(self.c_in, 128)
        self.mt = (self.c_out + 127) // 128
        self.pooled = i in POOL_AFTER
        self.h_out = h // 2 if self.pooled else h
        self.pw_out = self.h_out + 2  # fixed up below to next layer's pw


LC = [LayerCfg(i) for i in range(13)]
for _i in range(12):
    LC[_i].pw_out = LC[_i + 1].pw


# ---------------------------------------------------------------------------
# host-side parameter preparation
# ---------------------------------------------------------------------------

def _prep_params(params):
    f8np = mybir.dt.np(F8)
    out = {}
    for i in range(13):
        w = np.asarray(params["convs"][i], np.float32)
        g, b, m, v = [np.asarray(t, np.float32) for t in params["bns"][i]]
        scale = g / np.sqrt(v + np.float32(1e-5))
        shift = b - m * scale
        if i == 0:
            # lhsT [81, M]: fp22 split [wh; wh; wl] against rhs [xh; xl; xh]
            # gives near-fp32 products in one float32r pass.
            # k within each third = c*9 + dy*3 + dx
            lhsT = w.transpose(1, 2, 3, 0).reshape(27, 64).astype(np.float32)
            if USE_DUP:
                lhsT = np.concatenate([lhsT, lhsT], axis=1)  # [27, 128]
            wh = _trunc11(lhsT)
            wl = lhsT - wh
            out["w0"] = np.ascontiguousarray(np.concatenate([wh, wh, wl], 0))
            ab = np.stack([scale, shift])
            out["ab0"] = np.concatenate([ab, ab], 1) if USE_DUP else ab
        else:
            lc = LC[i]
            alpha = np.mean(np.abs(w), axis=(1, 2, 3), dtype=np.float32)
            A = scale * np.minimum(alpha, np.float32(1.0))
            assert np.all(A >= 0), "pool/sign commute needs A >= 0"
            ab = np.stack([A, shift])
            if USE_DUP and i == 1:
                ab = np.concatenate([ab, ab], 1)  # replicated on dup half
            out[f"ab{i}"] = ab
            ws = np.sign(w).astype(np.float32)  # [O, C, 3, 3]
            C, O = lc.c_in, lc.c_out
            if USE_DUP and i in (1, 2):
                # dup-format lhsT [128, 6, 1, O]: slots 0-2 pair (dy, dx=-1)
                # on rows 0:64 with (dy, dx=0) on rows 64:128 (the shifted
                # dup half); slots 3-5 are the dx=+1 singles (upper rows 0).
                wp = np.zeros((128, 6, 1, O), np.float32)
                for s in range(3):
                    wp[0:64, s, 0] = ws[:, :, s, 0].T  # dx=-1, dy=s-1
                    wp[64:128, s, 0] = ws[:, :, s, 1].T  # dx=0
                    wp[0:64, s + 3, 0] = ws[:, :, s, 2].T  # dx=+1
            else:
                wst = ws.transpose(1, 2, 3, 0).reshape(C, 9, O)  # [C, t, O]
                if lc.c_in >= 128:
                    wp = wst.reshape(lc.kg, 128, 9, O).transpose(1, 2, 0, 3)
                else:
                    wp = wst.reshape(1, C, 9, O).transpose(1, 2, 0, 3)
            # [P, 9|6, KG, O]
            out[f"w{i}"] = np.ascontiguousarray(wp).astype(f8np)
    fc_w = np.asarray(params["fc_w"], np.float32)  # [10, 512]
    fc_b = np.asarray(params["fc_b"], np.float32)
    out["fcw"] = np.ascontiguousarray(
        fc_w.T.reshape(4, 128, NUM_CLASSES).transpose(1, 0, 2)
    )  # [128, 4, 10]
    out["fcb"] = fc_b
    return out


def _trunc11(v):
    """Truncate fp32 mantissa to 11 bits (survives the PE's fp22 cast)."""
    return (v.view(np.uint32) & np.uint32(0xFFFFF000)).view(np.float32)


def _im2col(x):
    n = x.shape[0]
    xp = np.zeros((n, 3, 226, 226), np.float32)
    xp[:, :, 1:225, 1:225] = x
    im = np.empty((n, 81, 224 * 224), np.float32)
    for c in range(3):
        for dy in range(3):
            for dx in range(3):
                v = xp[:, c, dy : dy + 224, dx : dx + 224].reshape(n, -1)
                vh = _trunc11(v)
                k = c * 9 + dy * 3 + dx
                im[:, k] = vh
                im[:, 27 + k] = v - vh
                im[:, 54 + k] = vh
    return im


# ---------------------------------------------------------------------------
# device program
# ---------------------------------------------------------------------------

def _emit_borders(nc, buf, pw, h_out, bs, kg):
    """Zero the pad ring of one act buffer (data at offset +16).

    Rows are pw wide but only cols 1..w (w = h_out) hold data; cols 0 and
    w+1..pw-1 (right border + stride padding) must read as zero. Interior
    positions touched here are rewritten by the later sign-pass writes
    (memsets are emitted first, so ordering is safe), while never-written
    junk spans get zeroed -- keeping every matmul rhs read NaN-free.
    """
    w = h_out
    for g in range(kg):
        # front pad + row0 + row1's left col
        nc.vector.memset(buf[:, g, 0 : 16 + pw + 1], 0.0)
        # per row r=1..h: cols w..pw-1, plus col 0 of row r+1
        v = buf[:, g, 16 + pw + w : 16 + pw + w + h_out * pw]
        v = v.rearrange("p (r c) -> p r c", r=h_out, c=pw)
        nc.vector.memset(v[:, :, 0 : pw - w + 1], 0.0)
        # bottom row + tail pad
        nc.vector.memset(buf[:, g, 16 + (h_out + 1) * pw + 1 : bs], 0.0)


def build_nc():
    nc = bacc.Bacc("TRN2")
    d_x0 = nc.declare_dram_parameter(
        "x0", [IMGS_PER_CORE, 81, 224 * 224], F32R, isOutput=False
    )
    m0 = 128 if USE_DUP else 64
    d_w = {0: nc.declare_dram_parameter("w0", [81, m0], F32R, isOutput=False)}
    d_ab = {}
    for i in range(13):
        lc = LC[i]
        if i > 0:
            if USE_DUP and i in (1, 2):
                d_w[i] = nc.declare_dram_parameter(
                    f"w{i}", [128, 6, 1, lc.c_out], F8, isOutput=False)
            else:
                d_w[i] = nc.declare_dram_parameter(
                    f"w{i}", [lc.p_in, 9, lc.kg, lc.c_out], F8, isOutput=False)
        nab = 2 * lc.c_out if (USE_DUP and i in (0, 1)) else lc.c_out
        d_ab[i] = nc.declare_dram_parameter(
            f"ab{i}", [2, nab], F32, isOutput=False
        )
    d_fcw = nc.declare_dram_parameter("fcw", [128, 4, NUM_CLASSES], F32,
                                      isOutput=False)
    d_fcb = nc.declare_dram_parameter("fcb", [NUM_CLASSES], F32, isOutput=False)
    d_out = nc.declare_dram_parameter("out", [IMGS_PER_CORE, NUM_CLASSES], F32,
                                      isOutput=True)
    dbg_i = int(os.environ.get("DEBUG_ACT", "-1"))
    d_dbg = None
    if dbg_i >= 0:
        nxt = LC[dbg_i + 1]
        p = 128 if (USE_DUP and dbg_i in (0, 1)) else nxt.p_in
        d_dbg = nc.declare_dram_parameter(
            "dbg", [p, nxt.kg * nxt.bs], F8, isOutput=True)

    with tile.TileContext(nc) as tc:
        with (
            tc.tile_pool(name="wpool", bufs=2) as wpool,
            tc.tile_pool(name="abpool", bufs=1) as abpool,
            tc.tile_pool(name="iopool", bufs=4) as iopool,
            tc.tile_pool(name="tmppool", bufs=4) as tmppool,
            tc.tile_pool(name="pspool", bufs=8, space="PSUM") as pspool,
        ):
            # per-channel A/B columns, persistent (tiny)
            ab_sb = {}
            for i in range(13):
                lc = LC[i]
                nch = 2 * lc.c_out if (USE_DUP and i in (0, 1)) else lc.c_out
                t = abpool.tile([min(128, nch), 2 * lc.mt], F32,
                                name=f"ab_sb{i}")
                for m in range(lc.mt):
                    sl = slice(128 * m, 128 * m + min(128, nch - 128 * m))
                    nc.sync.dma_start(out=t[:, 2 * m], in_=d_ab[i][0, sl])
                    nc.sync.dma_start(out=t[:, 2 * m + 1], in_=d_ab[i][1, sl])
                ab_sb[i] = t
            fcw_sb = abpool.tile([128, 4, NUM_CLASSES], F32, name="fcw_sb")
            nc.sync.dma_start(out=fcw_sb[:], in_=d_fcw[:])
            fcb_sb = abpool.tile([NUM_CLASSES, 1], F32, name="fcb_sb")
            nc.sync.dma_start(out=fcb_sb[:, 0], in_=d_fcb[:])
            w0_sb = abpool.tile([81, m0], F32R, name="w0_sb")
            nc.sync.dma_start(out=w0_sb[:], in_=d_w[0][:])

            for img in range(IMGS_PER_CORE):
                with tc.tile_pool(name=f"acts{img}", bufs=1) as acts:
                    bufs = {}
                    for i in range(13):
                        lc = LC[i]
                        if i < 12:
                            nxt = LC[i + 1]
                            p = 128 if (USE_DUP and i in (0, 1)) else nxt.p_in
                            nblk = nxt.kg + (1 if nxt.tp else 0)
                            bufs[i] = acts.tile(
                                [p, nblk, nxt.bs], F8,
                                name=f"act{img}_{i}",
                            )
                            _emit_borders(nc, bufs[i], nxt.pw, nxt.h, nxt.bs,
                                          nxt.kg)
                            if nxt.tp:
                                # shifted block: front pad + tail (past the
                                # copied span) must read as zero
                                nc.vector.memset(bufs[i][:, nxt.kg, 0:16],
                                                 0.0)
                                nc.vector.memset(
                                    bufs[i][:, nxt.kg,
                                            16 + nxt.s - 2 * nxt.pw :
                                            nxt.bs], 0.0)

                    # ---- layer 0: fp32 im2col conv + BN + sign
                    lc = LC[0]
                    out_buf = bufs[0]
                    for r0 in range(0, 224, 2):
                        rhs = iopool.tile([81, 448], F32R, tag="l0rhs", bufs=3)
                        nc.sync.dma_start(
                            out=rhs[:],
                            in_=d_x0[img, :, r0 * 224 : (r0 + 2) * 224],
                        )
                        ps = pspool.tile([m0, 512], F32, tag="ps", bufs=8)
                        nc.tensor.matmul(ps[:, 0:448], w0_sb[:], rhs[:],
                                         start=True, stop=True)
                        st = 16 + (r0 + 1) * 226 + 1
                        dst = out_buf[0:64, 0, st : st + 2 * 226]
                        dst = dst.rearrange("p (r c) -> p r c", r=2, c=226)
                        nc.scalar.activation(
                            dst[:, :, 0:224],
                            ps[0:64, 0:448].rearrange("p (r c) -> p r c", r=2,
                                                      c=224),
                            mybir.ActivationFunctionType.Sign,
                            bias=ab_sb[0][0:64, 1:2],
                            scale=ab_sb[0][0:64, 0:1],
                        )
                        if USE_DUP:
                            # dup half: same values, shifted left by one,
                            # on partitions 64:128 (channels duplicated in w0)
                            dst2 = out_buf[64:128, 0, st - 1 : st - 1 + 452]
                            dst2 = dst2.rearrange("p (r c) -> p r c", r=2,
                                                  c=226)
                            nc.scalar.activation(
                                dst2[:, :, 0:224],
                                ps[64:128, 0:448].rearrange(
                                    "p (r c) -> p r c", r=2, c=224),
                                mybir.ActivationFunctionType.Sign,
                                bias=ab_sb[0][64:128, 1:2],
                                scale=ab_sb[0][64:128, 0:1],
                            )

                    # ---- layers 1..12: binary convs
                    for i in range(1, 13):
                        lc = LC[i]
                        dup_in = USE_DUP and i in (1, 2)
                        nslots = 6 if dup_in else 9
                        wp_part = 128 if dup_in else lc.p_in
                        w_t = wpool.tile([wp_part, nslots, lc.kg, lc.c_out],
                                         F8, tag="w", name=f"w_sb{img}_{i}")
                        nc.sync.dma_start(out=w_t[:], in_=d_w[i][:])
                        in_buf = bufs[i - 1]
                        out_buf = bufs[i] if i < 12 else None
                        feat = None
                        if i == 12:
                            feat = tmppool.tile([128, 4], F32, tag="feat",
                                                name=f"feat{img}")

                        def fill_shift_block(_i=i):
                            # block1[f] = block0[f + 2*pw] for next layer's
                            # vertical DoubleRow tap pairs (4 chunked DMAs)
                            nxt2 = LC[_i + 1]
                            span = nxt2.s - 2 * nxt2.pw
                            ck = ((span + 3) // 4 + 15) & ~15
                            for c0 in range(0, span, ck):
                                c1 = min(span, c0 + ck)
                                nc.sync.dma_start(
                                    out=bufs[_i][:, nxt2.kg,
                                                 16 + c0 : 16 + c1],
                                    in_=bufs[_i][:, 0,
                                                 16 + 2 * nxt2.pw + c0 :
                                                 16 + 2 * nxt2.pw + c1])

                        if dup_in:
                            taps = [(s, (s - 1) * lc.pw - 1) for s in range(3)]
                            taps += [(s, (s - 4) * lc.pw + 1)
                                     for s in range(3, 6)]
                        else:
                            taps = [((dy + 1) * 3 + dx + 1, dy * lc.pw + dx)
                                    for dy in (-1, 0, 1) for dx in (-1, 0, 1)]

                        def conv_mms(ps_ap, f0, n, msl, _lc=lc, _w=w_t,
                                     _in=in_buf, _taps=taps, _dup=dup_in):
                            mms = []
                            if _dup and _lc.tp:
                                # vertical pair via the shifted dup block:
                                # block1[f] = block0[f + 2*pw]
                                b = 16 + f0 - _lc.pw - 1
                                mms.append((_w[:, 0:3:2, 0, msl],
                                            _in[:, 0:2, b : b + n], DR))
                                mms.append((_w[:, 1, 0, msl],
                                            _in[:, 0, 16 + f0 - 1 :
                                                16 + f0 - 1 + n], None))
                                b = 16 + f0 - _lc.pw + 1
                                mms.append((_w[:, 3:6:2, 0, msl],
                                            _in[:, 0:2, b : b + n], DR))
                                mms.append((_w[:, 4, 0, msl],
                                            _in[:, 0, 16 + f0 + 1 :
                                                16 + f0 + 1 + n], None))
                            elif (not _dup) and _lc.kg == 1 and _lc.tp:
                                for dx in (-1, 0, 1):
                                    b = 16 + f0 - _lc.pw + dx
                                    mms.append((_w[:, dx + 1 : dx + 8 : 6, 0,
                                                   msl],
                                                _in[:, 0:2, b : b + n], DR))
                                for dx in (-1, 0, 1):
                                    b = 16 + f0 + dx
                                    mms.append((_w[:, 4 + dx, 0, msl],
                                                _in[:, 0, b : b + n], None))
                            else:
                                for (slot, off) in _taps:
                                    base = 16 + f0 + off
                                    kg = 0
                                    while kg < _lc.kg:
                                        if USE_DR and _lc.kg - kg >= 2:
                                            mms.append(
                                                (_w[:, slot, kg : kg + 2,
                                                    msl],
                                                 _in[:, kg : kg + 2,
                                                     base : base + n], DR))
                                            kg += 2
                                        else:
                                            mms.append(
                                                (_w[:, slot, kg, msl],
                                                 _in[:, kg, base : base + n],
                                                 None))
                                            kg += 1
                            for j, (lw, rh, pm) in enumerate(mms):
                                nc.tensor.matmul(
                                    ps_ap, lw, rh, start=(j == 0),
                                    stop=(j == len(mms) - 1), perf_mode=pm)

                        if USE_DUP and i == 1:
                            # column-packed: two row-pair tiles per PSUM tile
                            # (A -> psum[0:64] = odd pooled rows -> primary,
                            #  B -> psum[64:128] = even pooled rows -> dup)
                            pw2 = lc.pw_out
                            w2 = lc.h // 2
                            npair = 2 * lc.pw
                            for tp in range(lc.h // 4):
                                rA = 4 * tp
                                rB = rA + 2
                                ps = pspool.tile([128, 512], F32, tag="ps",
                                                 bufs=8)
                                # interleave A/B matmuls so consecutive MMs
                                # hit different col-groups (subarray overlap)
                                for si, (slot, off) in enumerate(taps):
                                    for hx, rh in ((0, rA), (1, rB)):
                                        sl = slice(64 * hx, 64 * hx + 64)
                                        base = 16 + (rh + 1) * lc.pw + off
                                        nc.tensor.matmul(
                                            ps[sl, 0:npair],
                                            w_t[:, slot, 0, 0:64],
                                            in_buf[:, 0, base : base + npair],
                                            start=(si == 0),
                                            stop=(si == len(taps) - 1))
                                # sign the whole tile at once, then pool +-1s
                                sgn = tmppool.tile([128, npair], F32,
                                                   tag="poolS", bufs=3)
                                nc.scalar.activation(
                                    sgn[:], ps[:, 0:npair],
                                    mybir.ActivationFunctionType.Sign,
                                    bias=ab_sb[1][:, 1:2],
                                    scale=ab_sb[1][:, 0:1])
                                rmax = tmppool.tile([128, lc.pw], F32,
                                                    tag="poolB", bufs=3)
                                nc.vector.tensor_tensor(
                                    rmax[:], sgn[:, 0 : lc.pw],
                                    sgn[:, lc.pw : npair],
                                    mybir.AluOpType.max)
                                cmax = tmppool.tile([128, w2], F32,
                                                    tag="poolC", bufs=3)
                                nc.vector.tensor_tensor(
                                    cmax[:],
                                    rmax[:, 1 : 1 + 2 * w2 : 2],
                                    rmax[:, 2 : 2 + 2 * w2 : 2],
                                    mybir.AluOpType.max)
                                pA = rA // 2 + 1
                                stA = 16 + pA * pw2 + 1
                                nc.scalar.copy(
                                    out_buf[0:64, 0, stA : stA + w2],
                                    cmax[0:64, :])
                                stB = 16 + (pA + 1) * pw2
                                nc.vector.tensor_copy(
                                    out_buf[64:128, 0, stB : stB + w2],
                                    cmax[64:128, :])
                            # complement copies to finish the dup format
                            nrow = w2 // 2
                            # primary even rows <- dup even rows (shift -1)
                            st_d = 16 + 2 * pw2
                            v_src = out_buf[64:128, 0,
                                            st_d : st_d + nrow * 2 * pw2]
                            v_src = v_src.rearrange("p (r c) -> p r c",
                                                    r=nrow, c=2 * pw2)
                            v_dst = out_buf[0:64, 0,
                                            st_d + 1 : st_d + 1
                                            + nrow * 2 * pw2]
                            v_dst = v_dst.rearrange("p (r c) -> p r c",
                                                    r=nrow, c=2 * pw2)
                            nc.sync.dma_start(out=v_dst[:, :, 0:w2],
                                              in_=v_src[:, :, 0:w2])
                            # dup odd rows <- primary odd rows (shift +1)
                            st_p = 16 + pw2
                            u_src = out_buf[0:64, 0,
                                            st_p + 1 : st_p + 1
                                            + nrow * 2 * pw2]
                            u_src = u_src.rearrange("p (r c) -> p r c",
                                                    r=nrow, c=2 * pw2)
                            u_dst = out_buf[64:128, 0,
                                            st_p : st_p + nrow * 2 * pw2]
                            u_dst = u_dst.rearrange("p (r c) -> p r c",
                                                    r=nrow, c=2 * pw2)
                            nc.sync.dma_start(out=u_dst[:, :, 0:w2],
                                              in_=u_src[:, :, 0:w2])
                            if LC[i + 1].tp:
                                fill_shift_block()
                            continue

                        rpt = ROWS_PER_TILE[i]
                        r0 = 0
                        while r0 < lc.h:
                            rr = min(rpt, lc.h - r0)
                            f0 = (r0 + 1) * lc.pw
                            n = rr * lc.pw
                            for m in range(lc.mt):
                                mw = min(128, lc.c_out - 128 * m)
                                msl = slice(128 * m, 128 * m + mw)
                                ps = pspool.tile([mw, 512], F32, tag="ps",
                                                 bufs=8)
                                conv_mms(ps[:, 0:n], f0, n, msl)
                                if lc.pooled:
                                    # sign the whole psum tile, then 2x2 max
                                    # of +-1 values (matches reference order)
                                    half = rr // 2
                                    w2 = lc.h // 2
                                    sgn = tmppool.tile([mw, n], F32,
                                                       tag="poolS", bufs=3)
                                    nc.scalar.activation(
                                        sgn[:], ps[:, 0:n],
                                        mybir.ActivationFunctionType.Sign,
                                        bias=ab_sb[i][:, 2 * m + 1 :
                                                      2 * m + 2],
                                        scale=ab_sb[i][:, 2 * m : 2 * m + 1])
                                    sg3 = sgn[:].rearrange(
                                        "p (r c) -> p r c", r=rr, c=lc.pw)
                                    rmax = tmppool.tile(
                                        [mw, half * lc.pw], F32, tag="poolB",
                                        bufs=3)
                                    nc.vector.tensor_tensor(
                                        rmax[:].rearrange(
                                            "p (r c) -> p r c", r=half,
                                            c=lc.pw),
                                        sg3[:, 0 : rr : 2, :],
                                        sg3[:, 1 : rr : 2, :],
                                        mybir.AluOpType.max)
                                    rm3 = rmax[:].rearrange(
                                        "p (r c) -> p r c", r=half, c=lc.pw)
                                    if i == 12:
                                        cmax = tmppool.tile(
                                            [mw, half * w2], F32, tag="poolC",
                                            bufs=3)
                                        nc.vector.tensor_tensor(
                                            cmax[:].rearrange(
                                                "p (r c) -> p r c", r=half,
                                                c=w2),
                                            rm3[:, :, 1 : 1 + 2 * w2 : 2],
                                            rm3[:, :, 2 : 2 + 2 * w2 : 2],
                                            mybir.AluOpType.max)
                                        nc.vector.reduce_sum(
                                            feat[:, m : m + 1], cmax[:],
                                            axis=mybir.AxisListType.X)
                                    else:
                                        pw2 = lc.pw_out
                                        st = 16 + (r0 // 2 + 1) * pw2 + 1
                                        dst = out_buf[:, m,
                                                      st : st + half * pw2]
                                        nc.vector.tensor_tensor(
                                            dst.rearrange(
                                                "p (r c) -> p r c", r=half,
                                                c=pw2)[:, :, 0:w2],
                                            rm3[:, :, 1 : 1 + 2 * w2 : 2],
                                            rm3[:, :, 2 : 2 + 2 * w2 : 2],
                                            mybir.AluOpType.max)
                                else:
                                    # interior-only write (borders stay zero)
                                    pwo = lc.pw_out
                                    st = 16 + (r0 + 1) * pwo + 1
                                    dst = out_buf[:, m, st : st + rr * pwo]
                                    dst = dst.rearrange(
                                        "p (r c) -> p r c", r=rr, c=pwo)
                                    src = ps[:, 0:n].rearrange(
                                        "p (r c) -> p r c", r=rr, c=lc.pw)
                                    nc.scalar.activation(
                                        dst[:, :, 0 : lc.h],
                                        src[:, :, 1 : 1 + lc.h],
                                        mybir.ActivationFunctionType.Sign,
                                        bias=ab_sb[i][:, 2 * m + 1 : 2 * m + 2],
                                        scale=ab_sb[i][:, 2 * m : 2 * m + 1])
                            r0 += rr
                        if i < 12 and LC[i + 1].tp:
                            fill_shift_block()

                    # ---- head: FC on accumulated sign sums (feat = 49*mean)
                    ps_fc = pspool.tile([NUM_CLASSES, 512], F32, tag="ps",
                                        bufs=8)
                    for kg in range(4):
                        nc.tensor.matmul(
                            ps_fc[:, 0:1], fcw_sb[:, kg, :],
                            feat[:, kg : kg + 1],
                            start=(kg == 0), stop=(kg == 3))
                    logits = tmppool.tile([NUM_CLASSES, 1], F32, tag="logits",
                                          bufs=2)
                    nc.scalar.activation(
                        logits[:], ps_fc[:, 0:1],
                        mybir.ActivationFunctionType.Identity,
                        bias=fcb_sb[:], scale=float(np.float32(1.0 / 49.0)))
                    nc.sync.dma_start(out=d_out[img, :], in_=logits[:, 0])
                    if d_dbg is not None and img == 0:
                        b = bufs[dbg_i]
                        nc.sync.dma_start(
                            out=d_dbg[:],
                            in_=b[:].rearrange("p a b -> p (a b)"))

    nc.finalize()
    return nc


_NC_CACHE = None


def kernel(x, params):
    global _NC_CACHE
    x = np.asarray(x, np.float32)
    prep = _prep_params(params)
    im = _im2col(x)
    if _NC_CACHE is None:
        _NC_CACHE = build_nc()
    nc = _NC_CACHE
    in_maps = []
    for core in range(N_CORES):
        m = dict(prep)
        m["x0"] = np.ascontiguousarray(
            im[core * IMGS_PER_CORE : (core + 1) * IMGS_PER_CORE])
        in_maps.append(m)
    res = run_bass_kernel_spmd(nc, in_maps, list(range(N_CORES))).results
    return np.concatenate([r["out"] for r in res], axis=0)
